# revision 15
# baseline (speedup 1.0000x reference)
"""Trainium2 Bass kernel for nn_BLayer_63780264346268 (topk_masking).

Math (per output unit o of 512):
  idx = top6(mask[o])                                  (6 of 1024 input features)
  h1 = relu(x[:, idx] @ W1[o, idx, :])                 (B,6)@(6,32)
  h2 = relu(h1 @ W2[o]); h3 = relu(h2 @ W3[o])         (B,32)@(32,32)
  y  = sigmoid(h3 @ W4[o]); q = (y>=.5)*2-1  == sign(h3 @ W4[o]) (as +/-1)

Distribution: 512 output units sharded across 8 cores (64 each). Top-k,
gathers and all math run on device; host does layout prep + final concat.

Per-core layout (64 units; o = 16i + 4m + j, i=strip, m=row-quadrant,
j=unit-in-quadrant; j1=j%2, j0=j//2):
  - top-8 values+indices per unit via DVE max/max_index; a 32x32-block
    DVE transpose + 4 stripe-replicated block DMAs build the int16 index
    tile for dma_gather (HW reads idx g from partition 16+g%16, col g//16;
    we replicate to all 8 stripes).
  - dma_gather x3: W1 block-diagonal lhsT per half (rows of a host-padded
    w1fp [65536, 128] where row o*1024+f holds W1[o,f,:] in column block
    32*(o%4)), and stageX from xT rows in one 512-row gather (row 1024 =
    zeros kills the j6=6,7 padding slots). An early dummy gather prepays
    the SWDGE first-use setup during the top-k window.
  - L1: per (m,i) one (K=32, M=128, N=256) matmul, 4 m-quadrants issued
    back-to-back on distinct PE row-groups (tile_position=(32m,0)).
  - L2/L3: block-diagonal (K=128, M=128, N=256) per quadrant t=4m+i.
  - L4: (K=128, M=4, N=256) per t on PE col-group m; Sign activation
    (W3 pre-scaled by |W4|, W4 reduced to signs) = the binarize.
  - PE is pre-warmed with dummy matmuls during the topk/gather head so the
    HAM clock gate sits at 2.4 GHz when the real matmuls arrive.
"""

import numpy as np

OUT, IN, HID, B = 512, 1024, 32, 256
NCORES = 8
OSH = OUT // NCORES  # 64 output units per core
HALF = 32 * IN  # w1fp rows per gather half (fits int16 indices)
WARM_MMS = 22  # PE warm-up matmuls riding the topk/gather head

_CACHE = {}


def _maps():
    # o = 16i + 4m + j ; mask/topk row r = 32*(j%2) + 8i + 2m + j//2
    o = np.arange(OSH)
    i, m, j = o // 16, (o % 16) // 4, o % 4
    r_of_o = 32 * (j % 2) + 8 * i + 2 * m + (j // 2)
    o_of_r = np.empty(OSH, np.int64)
    o_of_r[r_of_o] = o
    return o_of_r  # mask row r holds unit o_of_r[r]


def _build_program():
    import concourse.bacc as bacc
    import concourse.bass as bass
    import concourse.mybir as mybir
    import concourse.tile as tile

    f32 = mybir.dt.float32
    u16 = mybir.dt.uint16
    i16 = mybir.dt.int16
    RELU = mybir.ActivationFunctionType.Relu
    SIGN = mybir.ActivationFunctionType.Sign

    nc = bacc.Bacc(None, target_bir_lowering=False, debug=False)

    xT = nc.dram_tensor("xT", [IN + 1, B], f32, kind="ExternalInput")
    maskS = nc.dram_tensor("maskS", [OSH, IN], f32, kind="ExternalInput")
    oconst = nc.dram_tensor("oconst", [OSH, 1], u16, kind="ExternalInput")
    w1fp = nc.dram_tensor("w1fp", [OSH * IN, 4 * HID], f32, kind="ExternalInput")
    w2bd_d = nc.dram_tensor("w2bd", [128, 2048], f32, kind="ExternalInput")
    w3bd_d = nc.dram_tensor("w3bd", [128, 2048], f32, kind="ExternalInput")
    w4sgn = nc.dram_tensor("w4sgn", [128, 64], f32, kind="ExternalInput")
    outS = nc.dram_tensor("outS", [OSH, B], f32, kind="ExternalOutput")
    DBG = _CACHE.get("debug", False)
    if DBG:
        dbg_idx8 = nc.dram_tensor("dbg_idx8", [OSH, 8], u16, kind="ExternalOutput")
        dbg_idxs16 = nc.dram_tensor("dbg_idxs16", [128, 64], i16, kind="ExternalOutput")
        dbg_bd = nc.dram_tensor("dbg_bd", [128, 512], f32, kind="ExternalOutput")
        dbg_sx = nc.dram_tensor("dbg_sx", [128, 1024], f32, kind="ExternalOutput")
        dbg_h1s = nc.dram_tensor("dbg_h1s", [128, 4096], f32, kind="ExternalOutput")

    with tile.TileContext(nc) as tc:
        with (
            tc.tile_pool(name="const", bufs=1) as cpool,
            tc.tile_pool(name="psw", bufs=1, space="PSUM") as pswarm,
            tc.tile_pool(name="ps", bufs=5, space="PSUM") as pspool,
            tc.tile_pool(name="psy", bufs=1, space="PSUM") as psypool,
        ):
            # --- PE warm-up: dummy matmuls with no upstream deps ---
            warm = cpool.tile([128, 256], f32)
            nc.vector.memset(warm[:], 0.0)
            psw = pswarm.tile([128, 256], f32)
            for k in range(WARM_MMS):
                nc.tensor.matmul(
                    out=psw[:, 0:256],
                    lhsT=warm[:, 0:128],
                    rhs=warm[:, 0:256],
                    start=True,
                    stop=True,
                    tile_position=(0, 0),
                )

            # --- SWDGE first-use prepay: dummy gather on zero indices ---
            zidx = cpool.tile([128, 8], i16)
            nc.vector.memset(zidx[:], 0)
            gscratch = cpool.tile([128, 256], f32)
            nc.gpsimd.dma_gather(
                out_ap=gscratch[:].rearrange("p (i b) -> p i b", b=B),
                in_ap=xT[:, :],
                idxs_ap=zidx[:],
                num_idxs=128,
                num_idxs_reg=128,
                elem_size=B,
            )

            # --- loads ---
            mask_t = cpool.tile([OSH, IN], f32)
            nc.sync.dma_start(mask_t[0:32, :], maskS[0:32, :])
            nc.scalar.dma_start(mask_t[32:64, :], maskS[32:64, :])
            oconst_t = cpool.tile([OSH, 1], u16)
            nc.sync.dma_start(oconst_t[:], oconst[:])
            w4t = cpool.tile([128, 64], f32)
            nc.scalar.dma_start(w4t[:], w4sgn[:])
            w2bd = cpool.tile([128, 2048], f32)
            nc.scalar.dma_start(w2bd[:], w2bd_d[:])
            w3bd = cpool.tile([128, 2048], f32)
            nc.scalar.dma_start(w3bd[:], w3bd_d[:])

            # --- top-8 values + indices per unit (6 real, 2 padding) ---
            mx8 = cpool.tile([OSH, 8], f32)
            idx8 = cpool.tile([OSH, 8], u16)
            nc.vector.max(out=mx8[:], in_=mask_t[:])
            nc.vector.max_index(out=idx8[:], in_max=mx8[:], in_values=mask_t[:])

            # gwgx[64, 32] i16: cols 0:8 = x-row idx (pads -> zero row IN),
            # cols 8:16 = w1fp half-row idx (o%32)*1024 + idx, 16:32 unused.
            gwgx = cpool.tile([OSH, 32], i16)
            nc.vector.memset(gwgx[:, 16:32], 0)
            nc.vector.tensor_copy(gwgx[:, 0:6], idx8[:, 0:6])
            nc.vector.memset(gwgx[:, 6:8], IN)
            nc.vector.tensor_tensor(
                out=gwgx[:, 8:16],
                in0=idx8[:, 0:8],
                in1=oconst_t[:].to_broadcast([OSH, 8]),
                op=mybir.AluOpType.add,
            )
            # 32x32-block transpose: T[32*j1 + q, c] = gwgx[32*j1 + c, q]
            gwgxT = cpool.tile([OSH, 32], i16)
            nc.vector.transpose(gwgxT[:], gwgx[:])

            # idxs16[16k + 8j1 + j6, 32w + c] = T[32j1 + 8w + j6, c], all
            # stripes k (HW reads stripe 1, sim stripe 0 -- replicate).
            idxs16 = cpool.tile([128, 64], i16)
            for w in range(2):
                for j1 in range(2):
                    eng = nc.sync if w == 0 else nc.scalar
                    eng.dma_start(
                        out=idxs16[8 * j1 : 8 * j1 + 8, 32 * w : 32 * w + 32],
                        in_=gwgxT[32 * j1 + 8 * w : 32 * j1 + 8 * w + 8, :],
                    )
            # stripe replication by doubling: 16 -> 32 -> 64 -> 128 rows
            nc.sync.dma_start(idxs16[16:32, :], idxs16[0:16, :])
            nc.scalar.dma_start(idxs16[32:64, :], idxs16[0:32, :])
            nc.sync.dma_start(idxs16[64:128, :], idxs16[0:64, :])

            # --- gathers: W halves -> bd block-diag lhsT; X -> stageX ---
            bd = cpool.tile([128, 512], f32)
            stageX = cpool.tile([128, 1024], f32)

            def wgather(h):
                nc.gpsimd.dma_gather(
                    out_ap=bd[:, 256 * h : 256 * h + 256].rearrange(
                        "p (i q) -> p i q", q=128
                    ),
                    in_ap=w1fp[HALF * h : HALF * h + HALF, :],
                    idxs_ap=idxs16[:, 32 + 16 * h : 48 + 16 * h],
                    num_idxs=256,
                    num_idxs_reg=256,
                    elem_size=4 * HID,
                )

            wgather(0)
            nc.gpsimd.dma_gather(
                out_ap=stageX[:].rearrange("p (i b) -> p i b", b=B),
                in_ap=xT[:, :],
                idxs_ap=idxs16[:, 0:32],
                num_idxs=512,
                num_idxs_reg=512,
                elem_size=B,
            )
            wgather(1)

            def evac(dst, src, k):
                # alternate relu evacuation between Scalar and Vector
                if k % 2 == 0:
                    nc.scalar.activation(out=dst, in_=src, func=RELU)
                else:
                    nc.vector.tensor_scalar_max(dst, src, 0.0)

            # --- L1: per (m, i) one (K=32, M=128, N=256) matmul on PE
            # row-quadrant m; psum tile per (m, i-pair) wave ---
            h1s = cpool.tile([128, 4096], f32)
            nk_scalar = 0
            for w in range(2):  # wave = strip pair (0,1) then (2,3)
                for m in range(4):
                    ps1 = pspool.tile([128, 512], f32, tag="ps", name=f"ps1_{w}_{m}")
                    for ih in range(2):
                        i = 2 * w + ih
                        nc.tensor.matmul(
                            out=ps1[:, 256 * ih : 256 * ih + 256],
                            lhsT=bd[32 * m : 32 * m + 32, 128 * i : 128 * i + 128],
                            rhs=stageX[32 * m : 32 * m + 32, B * i : B * i + B],
                            start=True,
                            stop=True,
                            tile_position=(32 * m, 0),
                        )
                    # h1s quadrant t = 4m+i -> cols 256t; (m, wave) -> cols
                    # 1024m + 512w
                    evac(h1s[:, 1024 * m + 512 * w : 1024 * m + 512 * w + 512],
                         ps1[:, :], nk_scalar)
                    nk_scalar += 1

            # --- L2/L3: block-diag (K=128, M=128, N=256) per quadrant t;
            # L4 matmuls interleave into the L3 stream two pairs behind ---
            h2s = cpool.tile([128, 4096], f32)
            h3s = cpool.tile([128, 4096], f32)
            psy = psypool.tile([128, 1024], f32)
            nc.vector.memset(psy[:], 0.0)
            ys = cpool.tile([128, 1024], f32)
            TP_ORDER = [0, 2, 4, 6, 1, 3, 5, 7]  # wave-A-derived pairs first

            def l23(wt, hin, hout, tp):
                ps2 = pspool.tile([128, 512], f32, tag="ps", name=f"ps_{nk[0]}")
                for ih in range(2):
                    t = 2 * tp + ih
                    nc.tensor.matmul(
                        out=ps2[:, 256 * ih : 256 * ih + 256],
                        lhsT=wt[:, 128 * t : 128 * t + 128],
                        rhs=hin[:, B * t : B * t + B],
                        start=True,
                        stop=True,
                        tile_position=(0, 0),
                    )
                evac(hout[:, 512 * tp : 512 * tp + 512], ps2[:, :], nk[0])
                nk[0] += 1

            def l4pair(tp):
                for t in (2 * tp, 2 * tp + 1):
                    m, i = t // 4, t % 4
                    nc.tensor.matmul(
                        out=psy[32 * m : 32 * m + 4, 256 * i : 256 * i + 256],
                        lhsT=w4t[:, 4 * t : 4 * t + 4],
                        rhs=h3s[:, B * t : B * t + B],
                        start=True,
                        stop=True,
                        tile_position=(0, 32 * m),
                    )

            nk = [nk_scalar]
            for tp in TP_ORDER:
                l23(w2bd, h1s, h2s, tp)
            for k, tp in enumerate(TP_ORDER):
                l23(w3bd, h2s, h3s, tp)
                if k >= 2:
                    l4pair(TP_ORDER[k - 2])
            # psy half A (tps 0,2,4,6) is complete -> binarize while the
            # tail L4 pairs run
            nc.scalar.activation(out=ys[:, 0:512], in_=psy[:, 0:512], func=SIGN)
            for tp in (TP_ORDER[6], TP_ORDER[7]):
                l4pair(tp)
            nc.scalar.activation(out=ys[:, 512:1024], in_=psy[:, 512:1024], func=SIGN)

            if DBG:
                nc.sync.dma_start(dbg_idx8[:], idx8[:])
                nc.sync.dma_start(dbg_idxs16[:], idxs16[:])
                nc.sync.dma_start(dbg_bd[:], bd[:])
                nc.sync.dma_start(dbg_sx[:], stageX[:])
                nc.sync.dma_start(dbg_h1s[:], h1s[:])

            # outS[16i+4m+j, b] = ys[32m+j, 256i+b]; one DMA per quadrant m
            for m in range(4):
                eng = nc.sync if m % 2 == 0 else nc.scalar
                eng.dma_start(
                    out=outS[:].rearrange("(i mm j) b -> mm j i b", mm=4, j=4)[
                        m : m + 1
                    ],
                    in_=ys[32 * m : 32 * m + 4, :].rearrange("j (i b) -> j i b", b=B),
                )

    nc.compile()
    return nc


def _prep_core(c, inputs, mask, W1, W2, W3, W4, o_of_r):
    sl = slice(c * OSH, (c + 1) * OSH)
    mask_c = mask[sl]
    W1c, W2c, W3c, W4c = W1[sl], W2[sl], W3[sl], W4[sl]

    maskS = np.ascontiguousarray(mask_c[o_of_r])
    oconst = ((o_of_r.astype(np.uint16) % 32) * np.uint16(IN))[:, None]

    # w1fp[o*IN + f, 32*(o%4) + h] = W1c[o, f, h]
    w1fp = np.zeros((OSH, IN, 4, HID), np.float32)
    o = np.arange(OSH)
    w1fp[o, :, o % 4, :] = W1c
    w1fp = w1fp.reshape(OSH * IN, 4 * HID)

    # block-diag L2/L3 weights: col block t=4m+i holds lhsT for quadrant t:
    #   w2bd[32j+h, 128t + 32j+k] = W2c[o(t,j), h, k]
    #   w3bd[32j+k, 128t + 32j+l] = W3c[o,k,l] * |W4c[o,l]|
    w4v = W4c[:, :, 0]  # [64, 32]
    w3p = W3c * np.abs(w4v)[:, None, :]
    w2bd = np.zeros((128, 2048), np.float32)
    w3bd = np.zeros((128, 2048), np.float32)
    sgn = np.sign(w4v).astype(np.float32)
    w4sgn = np.zeros((128, 64), np.float32)
    for o in range(OSH):
        i, m, j = o // 16, (o % 16) // 4, o % 4
        t = 4 * m + i
        w2bd[32 * j : 32 * j + 32, 128 * t + 32 * j : 128 * t + 32 * j + 32] = W2c[o]
        w3bd[32 * j : 32 * j + 32, 128 * t + 32 * j : 128 * t + 32 * j + 32] = w3p[o]
        w4sgn[32 * j : 32 * j + 32, 4 * t + j] = sgn[o]

    return {
        "maskS": maskS.astype(np.float32),
        "oconst": oconst,
        "w1fp": w1fp,
        "w2bd": w2bd,
        "w3bd": w3bd,
        "w4sgn": w4sgn,
    }


def kernel(inputs, mask, W1, W2, W3, W4, _run_kwargs=None):
    from concourse.bass_utils import run_bass_kernel_spmd

    inputs = np.asarray(inputs, np.float32)
    mask = np.asarray(mask, np.float32)
    W1 = np.asarray(W1, np.float32)
    W2 = np.asarray(W2, np.float32)
    W3 = np.asarray(W3, np.float32)
    W4 = np.asarray(W4, np.float32)

    if "nc" not in _CACHE:
        _CACHE["nc"] = _build_program()
    nc = _CACHE["nc"]

    o_of_r = _maps()
    xT = np.zeros((IN + 1, B), np.float32)
    xT[:IN] = inputs.T
    in_maps = []
    for c in range(NCORES):
        m = _prep_core(c, inputs, mask, W1, W2, W3, W4, o_of_r)
        m["xT"] = xT
        in_maps.append(m)

    kw = dict(_run_kwargs or {})
    res = run_bass_kernel_spmd(nc, in_maps, core_ids=list(range(NCORES)), **kw)
    out = np.concatenate([r["outS"].T for r in res.results], axis=1)
    if _run_kwargs is not None:
        _CACHE["last_result"] = res
    return out.astype(np.float32)


# revision 18
# speedup vs baseline: 1.2116x; 1.2116x over previous
"""Trainium2 Bass kernel for nn_BLayer_63780264346268 (topk_masking).

Math (per output unit o of 512):
  idx = top6(mask[o])                                  (6 of 1024 input features)
  h1 = relu(x[:, idx] @ W1[o, idx, :])                 (B,6)@(6,32)
  h2 = relu(h1 @ W2[o]); h3 = relu(h2 @ W3[o])         (B,32)@(32,32)
  y  = sigmoid(h3 @ W4[o]); q = (y>=.5)*2-1  == sign(h3 @ W4[o]) (as +/-1)

Distribution: 512 output units sharded across 8 cores (64 each). Top-k,
gathers and all math run on device; host does layout prep + final concat.

Per-core layout (64 units; o = 16i + 4m + j, i=strip, m=row-quadrant,
j=unit-in-quadrant; j1=j%2, j0=j//2):
  - top-8 values+indices per unit via DVE max/max_index (u32 indices);
    per-strip flatten DMAs build [128,4] u32 offset tiles (partition
    32m+8j+j6, col i).
  - ONE indirect DMA with multi-offsets builds the W1 block-diagonal lhsT
    (rows of a host-padded w1fp [65536, 128] where row o*1024+f holds
    W1[o,f,:] in column block 32*(o%4)); ONE builds stageX from xT rows
    (row 1024 = zeros kills the j6=6,7 padding slots).
  - L1: per (m,i) one (K=32, M=128, N=256) matmul, 4 m-quadrants issued
    back-to-back on distinct PE row-groups (tile_position=(32m,0)).
  - L2/L3: block-diagonal (K=128, M=128, N=256) per quadrant t=4m+i.
  - L4: (K=128, M=4, N=256) per t on PE col-group m; Sign activation
    (W3 pre-scaled by |W4|, W4 reduced to signs) = the binarize.
  - PE is pre-warmed with dummy matmuls during the topk/gather head so the
    HAM clock gate sits at 2.4 GHz when the real matmuls arrive.
"""

import numpy as np

OUT, IN, HID, B = 512, 1024, 32, 256
NCORES = 8
OSH = OUT // NCORES  # 64 output units per core
WARM_MMS = 22  # PE warm-up matmuls riding the topk/gather head
MULTI_OFF = False  # HW probe: multi-offset indirect only honors partition 0

_CACHE = {}


def _maps():
    # o = 16i + 4m + j ; mask/topk row r = 16i + 4m + j (identity)
    return np.arange(OSH)


def _build_program():
    import concourse.bacc as bacc
    import concourse.bass as bass
    import concourse.mybir as mybir
    import concourse.tile as tile

    f32 = mybir.dt.float32
    f32r = mybir.dt.float32r
    u32 = mybir.dt.uint32
    RELU = mybir.ActivationFunctionType.Relu
    SIGN = mybir.ActivationFunctionType.Sign

    nc = bacc.Bacc(None, target_bir_lowering=False, debug=False)

    xT = nc.dram_tensor("xT", [IN + 1, B], f32, kind="ExternalInput")
    maskS = nc.dram_tensor("maskS", [OSH, IN], f32, kind="ExternalInput")
    oconst = nc.dram_tensor("oconst", [OSH, 1], u32, kind="ExternalInput")
    w1fp = nc.dram_tensor("w1fp", [OSH * IN, 4 * HID], f32, kind="ExternalInput")
    w2bd_d = nc.dram_tensor("w2bd", [128, 2048], f32, kind="ExternalInput")
    w3bd_d = nc.dram_tensor("w3bd", [128, 2048], f32, kind="ExternalInput")
    w4sgn = nc.dram_tensor("w4sgn", [128, 64], f32, kind="ExternalInput")
    outS = nc.dram_tensor("outS", [OSH, B], f32, kind="ExternalOutput")
    DBG = _CACHE.get("debug", False)
    if DBG:
        dbg_idx8 = nc.dram_tensor("dbg_idx8", [OSH, 8], u32, kind="ExternalOutput")
        dbg_offx = nc.dram_tensor("dbg_offx", [128, 4], u32, kind="ExternalOutput")
        dbg_bd = nc.dram_tensor("dbg_bd", [128, 512], f32, kind="ExternalOutput")
        dbg_sx = nc.dram_tensor("dbg_sx", [128, 1024], f32, kind="ExternalOutput")
        dbg_h1s = nc.dram_tensor("dbg_h1s", [128, 4096], f32, kind="ExternalOutput")

    with tile.TileContext(nc) as tc:
        with (
            tc.tile_pool(name="const", bufs=1) as cpool,
            tc.tile_pool(name="psw", bufs=1, space="PSUM") as pswarm,
            tc.tile_pool(name="ps", bufs=5, space="PSUM") as pspool,
            tc.tile_pool(name="psy", bufs=1, space="PSUM") as psypool,
        ):
            # --- PE warm-up: dummy matmuls with no upstream deps ---
            warm = cpool.tile([128, 256], f32)
            nc.vector.memset(warm[:], 0.0)
            psw = pswarm.tile([128, 256], f32)
            for k in range(WARM_MMS):
                nc.tensor.matmul(
                    out=psw[:, 0:256],
                    lhsT=warm[:, 0:128],
                    rhs=warm[:, 0:256],
                    start=True,
                    stop=True,
                    tile_position=(0, 0),
                )

            # --- loads ---
            mask_t = cpool.tile([OSH, IN], f32)
            nc.sync.dma_start(mask_t[0:32, :], maskS[0:32, :])
            nc.scalar.dma_start(mask_t[32:64, :], maskS[32:64, :])
            oconst_t = cpool.tile([OSH, 1], u32)
            nc.sync.dma_start(oconst_t[:], oconst[:])
            w4t = cpool.tile([128, 64], f32)
            nc.scalar.dma_start(w4t[:], w4sgn[:])
            w2bd = cpool.tile([128, 2048], f32)
            nc.scalar.dma_start(w2bd[:], w2bd_d[:])
            w3bd = cpool.tile([128, 2048], f32)
            nc.scalar.dma_start(w3bd[:], w3bd_d[:])

            # --- top-8 values + indices per unit (6 real, 2 padding) ---
            mx8 = cpool.tile([OSH, 8], f32)
            idx8 = cpool.tile([OSH, 8], u32)
            nc.vector.max(out=mx8[:], in_=mask_t[:])
            nc.vector.max_index(out=idx8[:], in_max=mx8[:], in_values=mask_t[:])

            # gx/gw [64, 8] u32: per-unit-row x-row idx (pads -> zero row
            # IN) and w1fp row idx o*1024 + idx.
            gx = cpool.tile([OSH, 8], u32)
            gw = cpool.tile([OSH, 8], u32)
            nc.vector.tensor_copy(gx[:, 0:6], idx8[:, 0:6])
            nc.vector.memset(gx[:, 6:8], IN)
            nc.vector.tensor_tensor(
                out=gw[:],
                in0=idx8[:],
                in1=oconst_t[:].to_broadcast([OSH, 8]),
                op=mybir.AluOpType.add,
            )

            # offset tiles [128, 1] per strip: off_i[32m+8j+j6] <-
            # g_[16i+4m+j, j6]; separate tiles keep the indirect-DMA
            # dependencies per-strip.
            offx = [
                cpool.tile([128, 1], u32, name=f"offx_{i}", tag=f"offx_{i}")
                for i in range(4)
            ]
            offw = [
                cpool.tile([128, 1], u32, name=f"offw_{i}", tag=f"offw_{i}")
                for i in range(4)
            ]
            for i in range(4):
                xeng = nc.sync if i % 2 == 0 else nc.scalar
                weng = nc.scalar if i % 2 == 0 else nc.sync
                xeng.dma_start(out=offx[i][:], in_=gx[16 * i : 16 * i + 16, 0:8])
                weng.dma_start(out=offw[i][:], in_=gw[16 * i : 16 * i + 16, 0:8])

            # --- indirect row gathers: W -> bd block-diag lhsT; X -> stageX
            bd = cpool.tile([128, 512], f32)
            stageX = cpool.tile([128, 1024], f32)

            def xgather(i):
                nc.gpsimd.indirect_dma_start(
                    out=stageX[:, B * i : B * i + B],
                    out_offset=None,
                    in_=xT[:, :],
                    in_offset=bass.IndirectOffsetOnAxis(ap=offx[i][:], axis=0),
                )

            def wgather(i):
                nc.gpsimd.indirect_dma_start(
                    out=bd[:, 128 * i : 128 * i + 128],
                    out_offset=None,
                    in_=w1fp[:, :],
                    in_offset=bass.IndirectOffsetOnAxis(ap=offw[i][:], axis=0),
                )

            for i in range(4):
                xgather(i)
                wgather(i)

            def evac(dst, src, k):
                # alternate relu evacuation between Scalar and Vector
                if k % 2 == 0:
                    nc.scalar.activation(out=dst, in_=src, func=RELU)
                else:
                    nc.vector.tensor_scalar_max(dst, src, 0.0)

            # --- L1 (per (m,i): K=32, M=128, N=256 on PE row-quadrant m)
            # interleaved with L2/L3 block-diag quadrant matmuls so the PE
            # stream never stalls on the strip-2/3 gathers; L4 matmuls ride
            # two pairs behind L3. ---
            h1s = cpool.tile([128, 4096], f32)
            h2s = cpool.tile([128, 4096], f32)
            h3s = cpool.tile([128, 4096], f32)
            psy = psypool.tile([128, 1024], f32)
            nc.vector.memset(psy[:], 0.0)
            ys = cpool.tile([128, 1024], f32)
            nk = [0]

            def l1wave(w):
                for m in range(4):
                    ps1 = pspool.tile([128, 512], f32, tag="ps", name=f"ps1_{w}_{m}")
                    for ih in range(2):
                        i = 2 * w + ih
                        nc.tensor.matmul(
                            out=ps1[:, 256 * ih : 256 * ih + 256],
                            lhsT=bd[32 * m : 32 * m + 32, 128 * i : 128 * i + 128],
                            rhs=stageX[32 * m : 32 * m + 32, B * i : B * i + B],
                            start=True,
                            stop=True,
                            tile_position=(32 * m, 0),
                        )
                    # h1s quadrant t = 4m+i at cols 256t
                    evac(h1s[:, 1024 * m + 512 * w : 1024 * m + 512 * w + 512],
                         ps1[:, :], nk[0])
                    nk[0] += 1

            R = _CACHE.get("f32r", False)

            def cast(ap):
                return ap.bitcast(f32r) if R else ap

            def l23(wt, hin, hout, tp):
                ps2 = pspool.tile([128, 512], f32, tag="ps", name=f"ps_{nk[0]}")
                for ih in range(2):
                    t = 2 * tp + ih
                    nc.tensor.matmul(
                        out=ps2[:, 256 * ih : 256 * ih + 256],
                        lhsT=cast(wt[:, 128 * t : 128 * t + 128]),
                        rhs=cast(hin[:, B * t : B * t + B]),
                        start=True,
                        stop=True,
                        tile_position=(0, 0),
                    )
                evac(hout[:, 512 * tp : 512 * tp + 512], ps2[:, :], nk[0])
                nk[0] += 1

            def l4pair(tp):
                for t in (2 * tp, 2 * tp + 1):
                    m, i = t // 4, t % 4
                    nc.tensor.matmul(
                        out=psy[32 * m : 32 * m + 4, 256 * i : 256 * i + 256],
                        lhsT=w4t[:, 4 * t : 4 * t + 4],
                        rhs=h3s[:, B * t : B * t + B],
                        start=True,
                        stop=True,
                        tile_position=(0, 32 * m),
                    )

            A_PAIRS = [0, 2, 4, 6]  # quadrant pairs fed by L1 wave A (i=0,1)
            B_PAIRS = [1, 3, 5, 7]
            l1wave(0)
            for tp in A_PAIRS:
                l23(w2bd, h1s, h2s, tp)
            l1wave(1)
            for tp in B_PAIRS:
                l23(w2bd, h1s, h2s, tp)
            order = A_PAIRS + B_PAIRS
            for k, tp in enumerate(order):
                l23(w3bd, h2s, h3s, tp)
                if k >= 2:
                    l4pair(order[k - 2])
            # psy half A (tps 0,2,4,6) complete -> binarize while tail L4 runs
            nc.scalar.activation(out=ys[:, 0:512], in_=psy[:, 0:512], func=SIGN)
            for tp in (order[6], order[7]):
                l4pair(tp)
            nc.scalar.activation(out=ys[:, 512:1024], in_=psy[:, 512:1024], func=SIGN)

            if DBG:
                nc.sync.dma_start(dbg_idx8[:], idx8[:])
                nc.sync.dma_start(dbg_offx[:], offx[:])
                nc.sync.dma_start(dbg_bd[:], bd[:])
                nc.sync.dma_start(dbg_sx[:], stageX[:])
                nc.sync.dma_start(dbg_h1s[:], h1s[:])

            # outS[16i+4m+j, b] = ys[32m+j, 256i+b]; one DMA per quadrant m
            for m in range(4):
                eng = nc.sync if m % 2 == 0 else nc.scalar
                eng.dma_start(
                    out=outS[:].rearrange("(i mm j) b -> mm j i b", mm=4, j=4)[
                        m : m + 1
                    ],
                    in_=ys[32 * m : 32 * m + 4, :].rearrange("j (i b) -> j i b", b=B),
                )

    nc.compile()
    return nc


def _prep_core(c, inputs, mask, W1, W2, W3, W4, o_of_r):
    sl = slice(c * OSH, (c + 1) * OSH)
    mask_c = mask[sl]
    W1c, W2c, W3c, W4c = W1[sl], W2[sl], W3[sl], W4[sl]

    maskS = np.ascontiguousarray(mask_c[o_of_r])
    oconst = (o_of_r.astype(np.uint32) * np.uint32(IN))[:, None]

    # w1fp[o*IN + f, 32*(o%4) + h] = W1c[o, f, h]
    w1fp = np.zeros((OSH, IN, 4, HID), np.float32)
    o = np.arange(OSH)
    w1fp[o, :, o % 4, :] = W1c
    w1fp = w1fp.reshape(OSH * IN, 4 * HID)

    # block-diag L2/L3 weights: col block t=4m+i holds lhsT for quadrant t:
    #   w2bd[32j+h, 128t + 32j+k] = W2c[o(t,j), h, k]
    #   w3bd[32j+k, 128t + 32j+l] = W3c[o,k,l] * |W4c[o,l]|
    w4v = W4c[:, :, 0]  # [64, 32]
    w3p = W3c * np.abs(w4v)[:, None, :]
    w2bd = np.zeros((128, 2048), np.float32)
    w3bd = np.zeros((128, 2048), np.float32)
    sgn = np.sign(w4v).astype(np.float32)
    w4sgn = np.zeros((128, 64), np.float32)
    for o in range(OSH):
        i, m, j = o // 16, (o % 16) // 4, o % 4
        t = 4 * m + i
        w2bd[32 * j : 32 * j + 32, 128 * t + 32 * j : 128 * t + 32 * j + 32] = W2c[o]
        w3bd[32 * j : 32 * j + 32, 128 * t + 32 * j : 128 * t + 32 * j + 32] = w3p[o]
        w4sgn[32 * j : 32 * j + 32, 4 * t + j] = sgn[o]

    return {
        "maskS": maskS.astype(np.float32),
        "oconst": oconst,
        "w1fp": w1fp,
        "w2bd": w2bd,
        "w3bd": w3bd,
        "w4sgn": w4sgn,
    }


def kernel(inputs, mask, W1, W2, W3, W4, _run_kwargs=None):
    from concourse.bass_utils import run_bass_kernel_spmd

    inputs = np.asarray(inputs, np.float32)
    mask = np.asarray(mask, np.float32)
    W1 = np.asarray(W1, np.float32)
    W2 = np.asarray(W2, np.float32)
    W3 = np.asarray(W3, np.float32)
    W4 = np.asarray(W4, np.float32)

    if "nc" not in _CACHE:
        _CACHE["nc"] = _build_program()
    nc = _CACHE["nc"]

    o_of_r = _maps()
    xT = np.zeros((IN + 1, B), np.float32)
    xT[:IN] = inputs.T
    in_maps = []
    for c in range(NCORES):
        m = _prep_core(c, inputs, mask, W1, W2, W3, W4, o_of_r)
        m["xT"] = xT
        in_maps.append(m)

    kw = dict(_run_kwargs or {})
    res = run_bass_kernel_spmd(nc, in_maps, core_ids=list(range(NCORES)), **kw)
    out = np.concatenate([r["outS"].T for r in res.results], axis=1)
    if _run_kwargs is not None:
        _CACHE["last_result"] = res
    return out.astype(np.float32)


# revision 20
# speedup vs baseline: 1.3493x; 1.1137x over previous
"""Trainium2 Bass kernel for nn_BLayer_63780264346268 (topk_masking).

Math (per output unit o of 512):
  idx = top6(mask[o])                                  (6 of 1024 input features)
  h1 = relu(x[:, idx] @ W1[o, idx, :])                 (B,6)@(6,32)
  h2 = relu(h1 @ W2[o]); h3 = relu(h2 @ W3[o])         (B,32)@(32,32)
  y  = sigmoid(h3 @ W4[o]); q = (y>=.5)*2-1  == sign(h3 @ W4[o]) (as +/-1)

Distribution: 512 output units sharded across 8 cores (64 each). Top-k,
gathers and all math run on device; host does layout prep + final concat.

Per-core layout (64 units; o = 16i + 4m + j, i=strip, m=row-quadrant,
j=unit-in-quadrant; j1=j%2, j0=j//2):
  - top-8 values+indices per unit via DVE max/max_index (u32 indices);
    per-strip flatten DMAs build [128,4] u32 offset tiles (partition
    32m+8j+j6, col i).
  - ONE indirect DMA with multi-offsets builds the W1 block-diagonal lhsT
    (rows of a host-padded w1fp [65536, 128] where row o*1024+f holds
    W1[o,f,:] in column block 32*(o%4)); ONE builds stageX from xT rows
    (row 1024 = zeros kills the j6=6,7 padding slots).
  - L1: per (m,i) one (K=32, M=128, N=256) matmul, 4 m-quadrants issued
    back-to-back on distinct PE row-groups (tile_position=(32m,0)).
  - L2/L3: block-diagonal (K=128, M=128, N=256) per quadrant t=4m+i.
  - L4: (K=128, M=4, N=256) per t on PE col-group m; Sign activation
    (W3 pre-scaled by |W4|, W4 reduced to signs) = the binarize.
  - PE is pre-warmed with dummy matmuls during the topk/gather head so the
    HAM clock gate sits at 2.4 GHz when the real matmuls arrive.
"""

import numpy as np

OUT, IN, HID, B = 512, 1024, 32, 256
NCORES = 8
OSH = OUT // NCORES  # 64 output units per core
WARM_MMS = 22  # PE warm-up matmuls riding the topk/gather head
MULTI_OFF = False  # HW probe: multi-offset indirect only honors partition 0

_CACHE = {}


def _maps():
    # o = 16i + 4m + j ; mask/topk row r = 16i + 4m + j (identity)
    return np.arange(OSH)


def _build_program():
    import concourse.bacc as bacc
    import concourse.bass as bass
    import concourse.mybir as mybir
    import concourse.tile as tile

    f32 = mybir.dt.float32
    f32r = mybir.dt.float32r
    u32 = mybir.dt.uint32
    RELU = mybir.ActivationFunctionType.Relu
    SIGN = mybir.ActivationFunctionType.Sign

    nc = bacc.Bacc(None, target_bir_lowering=False, debug=False)

    xT = nc.dram_tensor("xT", [IN + 1, B], f32, kind="ExternalInput")
    maskS = nc.dram_tensor("maskS", [OSH, IN], f32, kind="ExternalInput")
    oconst = nc.dram_tensor("oconst", [OSH, 1], u32, kind="ExternalInput")
    w1fp = nc.dram_tensor("w1fp", [OSH * IN, 4 * HID], f32, kind="ExternalInput")
    w2bd_d = nc.dram_tensor("w2bd", [128, 2048], f32, kind="ExternalInput")
    w3bd_d = nc.dram_tensor("w3bd", [128, 2048], f32, kind="ExternalInput")
    w4sgn = nc.dram_tensor("w4sgn", [128, 64], f32, kind="ExternalInput")
    outS = nc.dram_tensor("outS", [OSH, B], f32, kind="ExternalOutput")
    DBG = _CACHE.get("debug", False)
    if DBG:
        dbg_idx8 = nc.dram_tensor("dbg_idx8", [OSH, 8], u32, kind="ExternalOutput")
        dbg_offx = nc.dram_tensor("dbg_offx", [128, 4], u32, kind="ExternalOutput")
        dbg_bd = nc.dram_tensor("dbg_bd", [128, 512], f32, kind="ExternalOutput")
        dbg_sx = nc.dram_tensor("dbg_sx", [128, 1024], f32, kind="ExternalOutput")
        dbg_h1s = nc.dram_tensor("dbg_h1s", [128, 4096], f32, kind="ExternalOutput")

    with tile.TileContext(nc) as tc:
        with (
            tc.tile_pool(name="const", bufs=1) as cpool,
            tc.tile_pool(name="psw", bufs=1, space="PSUM") as pswarm,
            tc.tile_pool(name="ps", bufs=5, space="PSUM") as pspool,
            tc.tile_pool(name="psy", bufs=1, space="PSUM") as psypool,
        ):
            # --- PE warm-up: dummy matmuls with no upstream deps ---
            warm = cpool.tile([128, 256], f32)
            nc.vector.memset(warm[:], 0.0)
            psw = pswarm.tile([128, 256], f32)
            for k in range(WARM_MMS):
                nc.tensor.matmul(
                    out=psw[:, 0:256],
                    lhsT=warm[:, 0:128],
                    rhs=warm[:, 0:256],
                    start=True,
                    stop=True,
                    tile_position=(0, 0),
                )

            # --- loads ---
            mask_t = cpool.tile([OSH, IN], f32)
            nc.sync.dma_start(mask_t[0:32, :], maskS[0:32, :])
            nc.scalar.dma_start(mask_t[32:64, :], maskS[32:64, :])
            oconst_t = cpool.tile([OSH, 1], u32)
            nc.sync.dma_start(oconst_t[:], oconst[:])
            w4t = cpool.tile([128, 64], f32)
            nc.scalar.dma_start(w4t[:], w4sgn[:])
            w2bd = cpool.tile([128, 2048], f32)
            nc.scalar.dma_start(w2bd[:], w2bd_d[:])
            w3bd = cpool.tile([128, 2048], f32)
            nc.scalar.dma_start(w3bd[:], w3bd_d[:])

            # --- top-8 values + indices per unit (6 real, 2 padding) ---
            mx8 = cpool.tile([OSH, 8], f32)
            idx8 = cpool.tile([OSH, 8], u32)
            nc.vector.max(out=mx8[:], in_=mask_t[:])
            nc.vector.max_index(out=idx8[:], in_max=mx8[:], in_values=mask_t[:])

            # gx/gw [64, 8] u32: per-unit-row x-row idx (pads -> zero row
            # IN) and w1fp row idx o*1024 + idx.
            gx = cpool.tile([OSH, 8], u32)
            gw = cpool.tile([OSH, 8], u32)
            nc.vector.tensor_copy(gx[:, 0:6], idx8[:, 0:6])
            nc.vector.memset(gx[:, 6:8], IN)
            nc.vector.tensor_tensor(
                out=gw[:],
                in0=idx8[:],
                in1=oconst_t[:].to_broadcast([OSH, 8]),
                op=mybir.AluOpType.add,
            )

            # offset tiles [128, 1] per strip: off_i[32m+8j+j6] <-
            # g_[16i+4m+j, j6]; separate tiles keep the indirect-DMA
            # dependencies per-strip.
            offx = [
                cpool.tile([128, 1], u32, name=f"offx_{i}", tag=f"offx_{i}")
                for i in range(4)
            ]
            offw = [
                cpool.tile([128, 1], u32, name=f"offw_{i}", tag=f"offw_{i}")
                for i in range(4)
            ]
            for i in range(4):
                xeng = nc.sync if i % 2 == 0 else nc.scalar
                weng = nc.scalar if i % 2 == 0 else nc.sync
                xeng.dma_start(out=offx[i][:], in_=gx[16 * i : 16 * i + 16, 0:8])
                weng.dma_start(out=offw[i][:], in_=gw[16 * i : 16 * i + 16, 0:8])

            # --- indirect row gathers: W -> bd block-diag lhsT; X -> stageX
            bd = cpool.tile([128, 512], f32)
            stageX = cpool.tile([128, 1024], f32)

            def xgather(i):
                nc.gpsimd.indirect_dma_start(
                    out=stageX[:, B * i : B * i + B],
                    out_offset=None,
                    in_=xT[:, :],
                    in_offset=bass.IndirectOffsetOnAxis(ap=offx[i][:], axis=0),
                )

            def wgather(i):
                nc.gpsimd.indirect_dma_start(
                    out=bd[:, 128 * i : 128 * i + 128],
                    out_offset=None,
                    in_=w1fp[:, :],
                    in_offset=bass.IndirectOffsetOnAxis(ap=offw[i][:], axis=0),
                )

            for i in range(4):
                xgather(i)
                wgather(i)

            def evac(dst, src, k):
                # alternate relu evacuation between Scalar and Vector
                if k % 2 == 0:
                    nc.scalar.activation(out=dst, in_=src, func=RELU)
                else:
                    nc.vector.tensor_scalar_max(dst, src, 0.0)

            # --- L1 (per (m,i): K=32, M=128, N=256 on PE row-quadrant m)
            # interleaved with L2/L3 block-diag quadrant matmuls so the PE
            # stream never stalls on the strip-2/3 gathers; L4 matmuls ride
            # two pairs behind L3. ---
            h1s = cpool.tile([128, 4096], f32)
            h2s = cpool.tile([128, 4096], f32)
            h3s = cpool.tile([128, 4096], f32)
            psy = psypool.tile([128, 1024], f32)
            nc.vector.memset(psy[:], 0.0)
            ys = cpool.tile([128, 1024], f32)
            nk = [0]

            def l1wave(w):
                for m in range(4):
                    ps1 = pspool.tile([128, 512], f32, tag="ps", name=f"ps1_{w}_{m}")
                    for ih in range(2):
                        i = 2 * w + ih
                        nc.tensor.matmul(
                            out=ps1[:, 256 * ih : 256 * ih + 256],
                            lhsT=bd[32 * m : 32 * m + 32, 128 * i : 128 * i + 128],
                            rhs=stageX[32 * m : 32 * m + 32, B * i : B * i + B],
                            start=True,
                            stop=True,
                            tile_position=(32 * m, 0),
                        )
                    # h1s quadrant t = 4m+i at cols 256t
                    evac(h1s[:, 1024 * m + 512 * w : 1024 * m + 512 * w + 512],
                         ps1[:, :], nk[0])
                    nk[0] += 1

            R = _CACHE.get("f32r", False)

            def cast(ap):
                return ap.bitcast(f32r) if R else ap

            def l23(wt, hin, hout, tp):
                ps2 = pspool.tile([128, 512], f32, tag="ps", name=f"ps_{nk[0]}")
                for ih in range(2):
                    t = 2 * tp + ih
                    nc.tensor.matmul(
                        out=ps2[:, 256 * ih : 256 * ih + 256],
                        lhsT=cast(wt[:, 128 * t : 128 * t + 128]),
                        rhs=cast(hin[:, B * t : B * t + B]),
                        start=True,
                        stop=True,
                        tile_position=(0, 0),
                    )
                evac(hout[:, 512 * tp : 512 * tp + 512], ps2[:, :], nk[0])
                nk[0] += 1

            def l4quad(i):
                # 4 concurrent col-group matmuls (one per m) for strip i
                for m in range(4):
                    t = 4 * m + i
                    nc.tensor.matmul(
                        out=psy[32 * m : 32 * m + 4, 256 * i : 256 * i + 256],
                        lhsT=w4t[:, 4 * t : 4 * t + 4],
                        rhs=h3s[:, B * t : B * t + B],
                        start=True,
                        stop=True,
                        tile_position=(0, 32 * m),
                    )

            A_PAIRS = [0, 2, 4, 6]  # quadrant pairs fed by L1 wave A (i=0,1)
            B_PAIRS = [1, 3, 5, 7]
            l1wave(0)
            for tp in A_PAIRS:
                l23(w2bd, h1s, h2s, tp)
            l1wave(1)
            for tp in B_PAIRS:
                l23(w2bd, h1s, h2s, tp)
            # L3: A pairs produce h3 quadrants with t%4 in {0,1} (strips 0,1)
            for tp in A_PAIRS:
                l23(w3bd, h2s, h3s, tp)
            l23(w3bd, h2s, h3s, 1)
            l4quad(0)
            l4quad(1)
            nc.scalar.activation(out=ys[:, 0:512], in_=psy[:, 0:512], func=SIGN)
            for tp in (3, 5, 7):
                l23(w3bd, h2s, h3s, tp)
            l4quad(2)
            l4quad(3)
            nc.scalar.activation(out=ys[:, 512:1024], in_=psy[:, 512:1024], func=SIGN)

            if DBG:
                nc.sync.dma_start(dbg_idx8[:], idx8[:])
                nc.sync.dma_start(dbg_offx[:], offx[:])
                nc.sync.dma_start(dbg_bd[:], bd[:])
                nc.sync.dma_start(dbg_sx[:], stageX[:])
                nc.sync.dma_start(dbg_h1s[:], h1s[:])

            # outS[16i+4m+j, b] = ys[32m+j, 256i+b]; one DMA per
            # (m, strip-half) so half A ships while half B computes
            for h in range(2):
                for m in range(4):
                    eng = nc.sync if m % 2 == 0 else nc.scalar
                    eng.dma_start(
                        out=outS[32 * h : 32 * h + 32, :].rearrange(
                            "(i mm j) b -> mm j i b", mm=4, j=4
                        )[m : m + 1],
                        in_=ys[32 * m : 32 * m + 4, 512 * h : 512 * h + 512].rearrange(
                            "j (i b) -> j i b", b=B
                        ),
                    )

    nc.compile()
    return nc


def _prep_core(c, inputs, mask, W1, W2, W3, W4, o_of_r):
    sl = slice(c * OSH, (c + 1) * OSH)
    mask_c = mask[sl]
    W1c, W2c, W3c, W4c = W1[sl], W2[sl], W3[sl], W4[sl]

    maskS = np.ascontiguousarray(mask_c[o_of_r])
    oconst = (o_of_r.astype(np.uint32) * np.uint32(IN))[:, None]

    # w1fp[o*IN + f, 32*(o%4) + h] = W1c[o, f, h]
    w1fp = np.zeros((OSH, IN, 4, HID), np.float32)
    o = np.arange(OSH)
    w1fp[o, :, o % 4, :] = W1c
    w1fp = w1fp.reshape(OSH * IN, 4 * HID)

    # block-diag L2/L3 weights: col block t=4m+i holds lhsT for quadrant t:
    #   w2bd[32j+h, 128t + 32j+k] = W2c[o(t,j), h, k]
    #   w3bd[32j+k, 128t + 32j+l] = W3c[o,k,l] * |W4c[o,l]|
    w4v = W4c[:, :, 0]  # [64, 32]
    w3p = W3c * np.abs(w4v)[:, None, :]
    w2bd = np.zeros((128, 2048), np.float32)
    w3bd = np.zeros((128, 2048), np.float32)
    sgn = np.sign(w4v).astype(np.float32)
    w4sgn = np.zeros((128, 64), np.float32)
    for o in range(OSH):
        i, m, j = o // 16, (o % 16) // 4, o % 4
        t = 4 * m + i
        w2bd[32 * j : 32 * j + 32, 128 * t + 32 * j : 128 * t + 32 * j + 32] = W2c[o]
        w3bd[32 * j : 32 * j + 32, 128 * t + 32 * j : 128 * t + 32 * j + 32] = w3p[o]
        w4sgn[32 * j : 32 * j + 32, 4 * t + j] = sgn[o]

    return {
        "maskS": maskS.astype(np.float32),
        "oconst": oconst,
        "w1fp": w1fp,
        "w2bd": w2bd,
        "w3bd": w3bd,
        "w4sgn": w4sgn,
    }


def kernel(inputs, mask, W1, W2, W3, W4, _run_kwargs=None):
    from concourse.bass_utils import run_bass_kernel_spmd

    inputs = np.asarray(inputs, np.float32)
    mask = np.asarray(mask, np.float32)
    W1 = np.asarray(W1, np.float32)
    W2 = np.asarray(W2, np.float32)
    W3 = np.asarray(W3, np.float32)
    W4 = np.asarray(W4, np.float32)

    if "nc" not in _CACHE:
        _CACHE["nc"] = _build_program()
    nc = _CACHE["nc"]

    o_of_r = _maps()
    xT = np.zeros((IN + 1, B), np.float32)
    xT[:IN] = inputs.T
    in_maps = []
    for c in range(NCORES):
        m = _prep_core(c, inputs, mask, W1, W2, W3, W4, o_of_r)
        m["xT"] = xT
        in_maps.append(m)

    kw = dict(_run_kwargs or {})
    res = run_bass_kernel_spmd(nc, in_maps, core_ids=list(range(NCORES)), **kw)
    out = np.concatenate([r["outS"].T for r in res.results], axis=1)
    if _run_kwargs is not None:
        _CACHE["last_result"] = res
    return out.astype(np.float32)


# revision 21
# speedup vs baseline: 1.3753x; 1.0193x over previous
"""Trainium2 Bass kernel for nn_BLayer_63780264346268 (topk_masking).

Math (per output unit o of 512):
  idx = top6(mask[o])                                  (6 of 1024 input features)
  h1 = relu(x[:, idx] @ W1[o, idx, :])                 (B,6)@(6,32)
  h2 = relu(h1 @ W2[o]); h3 = relu(h2 @ W3[o])         (B,32)@(32,32)
  y  = sigmoid(h3 @ W4[o]); q = (y>=.5)*2-1  == sign(h3 @ W4[o]) (as +/-1)

Distribution: 512 output units sharded across 8 cores (64 each). Top-k,
gathers and all math run on device; host does layout prep + final concat.

Per-core layout (64 units; o = 16i + 4m + j, i=strip, m=row-quadrant,
j=unit-in-quadrant; j1=j%2, j0=j//2):
  - top-8 values+indices per unit via DVE max/max_index (u32 indices);
    per-strip flatten DMAs build [128,4] u32 offset tiles (partition
    32m+8j+j6, col i).
  - ONE indirect DMA with multi-offsets builds the W1 block-diagonal lhsT
    (rows of a host-padded w1fp [65536, 128] where row o*1024+f holds
    W1[o,f,:] in column block 32*(o%4)); ONE builds stageX from xT rows
    (row 1024 = zeros kills the j6=6,7 padding slots).
  - L1: per (m,i) one (K=32, M=128, N=256) matmul, 4 m-quadrants issued
    back-to-back on distinct PE row-groups (tile_position=(32m,0)).
  - L2/L3: block-diagonal (K=128, M=128, N=256) per quadrant t=4m+i.
  - L4: (K=128, M=4, N=256) per t on PE col-group m; Sign activation
    (W3 pre-scaled by |W4|, W4 reduced to signs) = the binarize.
  - PE is pre-warmed with dummy matmuls during the topk/gather head so the
    HAM clock gate sits at 2.4 GHz when the real matmuls arrive.
"""

import numpy as np

OUT, IN, HID, B = 512, 1024, 32, 256
NCORES = 8
OSH = OUT // NCORES  # 64 output units per core
WARM_MMS = 32  # PE warm-up matmuls riding the topk/gather head
MULTI_OFF = False  # HW probe: multi-offset indirect only honors partition 0

_CACHE = {}


def _maps():
    # o = 16i + 4m + j ; mask/topk row r = 16i + 4m + j (identity)
    return np.arange(OSH)


def _build_program():
    import concourse.bacc as bacc
    import concourse.bass as bass
    import concourse.mybir as mybir
    import concourse.tile as tile

    f32 = mybir.dt.float32
    f32r = mybir.dt.float32r
    u32 = mybir.dt.uint32
    RELU = mybir.ActivationFunctionType.Relu
    SIGN = mybir.ActivationFunctionType.Sign

    nc = bacc.Bacc(None, target_bir_lowering=False, debug=False)

    xT = nc.dram_tensor("xT", [IN + 1, B], f32, kind="ExternalInput")
    maskS = nc.dram_tensor("maskS", [OSH, IN], f32, kind="ExternalInput")
    oconst = nc.dram_tensor("oconst", [OSH, 1], u32, kind="ExternalInput")
    w1fp = nc.dram_tensor("w1fp", [OSH * IN, 4 * HID], f32, kind="ExternalInput")
    w2bd_d = nc.dram_tensor("w2bd", [128, 2048], f32, kind="ExternalInput")
    w3bd_d = nc.dram_tensor("w3bd", [128, 2048], f32, kind="ExternalInput")
    w4sgn = nc.dram_tensor("w4sgn", [128, 64], f32, kind="ExternalInput")
    outS = nc.dram_tensor("outS", [OSH, B], f32, kind="ExternalOutput")
    DBG = _CACHE.get("debug", False)
    if DBG:
        dbg_idx8 = nc.dram_tensor("dbg_idx8", [OSH, 8], u32, kind="ExternalOutput")
        dbg_offx = nc.dram_tensor("dbg_offx", [128, 4], u32, kind="ExternalOutput")
        dbg_bd = nc.dram_tensor("dbg_bd", [128, 512], f32, kind="ExternalOutput")
        dbg_sx = nc.dram_tensor("dbg_sx", [128, 1024], f32, kind="ExternalOutput")
        dbg_h1s = nc.dram_tensor("dbg_h1s", [128, 4096], f32, kind="ExternalOutput")

    with tile.TileContext(nc) as tc:
        with (
            tc.tile_pool(name="const", bufs=1) as cpool,
            tc.tile_pool(name="psw", bufs=1, space="PSUM") as pswarm,
            tc.tile_pool(name="ps", bufs=5, space="PSUM") as pspool,
            tc.tile_pool(name="psy", bufs=1, space="PSUM") as psypool,
        ):
            # --- PE warm-up: dummy matmuls with no upstream deps ---
            warm = cpool.tile([128, 256], f32)
            nc.vector.memset(warm[:], 0.0)
            psw = pswarm.tile([128, 256], f32)
            for k in range(WARM_MMS):
                nc.tensor.matmul(
                    out=psw[:, 0:256],
                    lhsT=warm[:, 0:128],
                    rhs=warm[:, 0:256],
                    start=True,
                    stop=True,
                    tile_position=(0, 0),
                )

            # --- loads ---
            mask_t = cpool.tile([OSH, IN], f32)
            nc.sync.dma_start(mask_t[0:32, :], maskS[0:32, :])
            nc.scalar.dma_start(mask_t[32:64, :], maskS[32:64, :])
            oconst_t = cpool.tile([OSH, 1], u32)
            nc.sync.dma_start(oconst_t[:], oconst[:])
            w4t = cpool.tile([128, 64], f32)
            nc.scalar.dma_start(w4t[:], w4sgn[:])
            w2bd = cpool.tile([128, 2048], f32)
            nc.scalar.dma_start(w2bd[:], w2bd_d[:])
            w3bd = cpool.tile([128, 2048], f32)
            nc.scalar.dma_start(w3bd[:], w3bd_d[:])

            # --- top-8 values + indices per unit (6 real, 2 padding) ---
            mx8 = cpool.tile([OSH, 8], f32)
            idx8 = cpool.tile([OSH, 8], u32)
            nc.vector.max(out=mx8[:], in_=mask_t[:])
            nc.vector.max_index(out=idx8[:], in_max=mx8[:], in_values=mask_t[:])

            # gx/gw [64, 8] u32: per-unit-row x-row idx (pads -> zero row
            # IN) and w1fp row idx o*1024 + idx.
            gx = cpool.tile([OSH, 8], u32)
            gw = cpool.tile([OSH, 8], u32)
            nc.vector.tensor_copy(gx[:, 0:6], idx8[:, 0:6])
            nc.vector.memset(gx[:, 6:8], IN)
            nc.vector.tensor_tensor(
                out=gw[:],
                in0=idx8[:],
                in1=oconst_t[:].to_broadcast([OSH, 8]),
                op=mybir.AluOpType.add,
            )

            # offset tiles [128, 1] per strip: off_i[32m+8j+j6] <-
            # g_[16i+4m+j, j6]; separate tiles keep the indirect-DMA
            # dependencies per-strip.
            offx = [
                cpool.tile([128, 1], u32, name=f"offx_{i}", tag=f"offx_{i}")
                for i in range(4)
            ]
            offw = [
                cpool.tile([128, 1], u32, name=f"offw_{i}", tag=f"offw_{i}")
                for i in range(4)
            ]
            for i in range(4):
                xeng = nc.sync if i % 2 == 0 else nc.scalar
                weng = nc.scalar if i % 2 == 0 else nc.sync
                xeng.dma_start(out=offx[i][:], in_=gx[16 * i : 16 * i + 16, 0:8])
                weng.dma_start(out=offw[i][:], in_=gw[16 * i : 16 * i + 16, 0:8])

            # --- indirect row gathers: W -> bd block-diag lhsT; X -> stageX
            bd = cpool.tile([128, 512], f32)
            stageX = cpool.tile([128, 1024], f32)

            def xgather(i):
                nc.gpsimd.indirect_dma_start(
                    out=stageX[:, B * i : B * i + B],
                    out_offset=None,
                    in_=xT[:, :],
                    in_offset=bass.IndirectOffsetOnAxis(ap=offx[i][:], axis=0),
                )

            def wgather(i):
                nc.gpsimd.indirect_dma_start(
                    out=bd[:, 128 * i : 128 * i + 128],
                    out_offset=None,
                    in_=w1fp[:, :],
                    in_offset=bass.IndirectOffsetOnAxis(ap=offw[i][:], axis=0),
                )

            for i in range(4):
                xgather(i)
                wgather(i)

            def evac(dst, src, k):
                # alternate relu evacuation between Scalar and Vector
                if k % 2 == 0:
                    nc.scalar.activation(out=dst, in_=src, func=RELU)
                else:
                    nc.vector.tensor_scalar_max(dst, src, 0.0)

            # --- L1 (per (m,i): K=32, M=128, N=256 on PE row-quadrant m)
            # interleaved with L2/L3 block-diag quadrant matmuls so the PE
            # stream never stalls on the strip-2/3 gathers; L4 matmuls ride
            # two pairs behind L3. ---
            h1s = cpool.tile([128, 4096], f32)
            h2s = cpool.tile([128, 4096], f32)
            h3s = cpool.tile([128, 4096], f32)
            psy = psypool.tile([128, 1024], f32)
            nc.vector.memset(psy[:], 0.0)
            ys = cpool.tile([128, 1024], f32)
            nk = [0]

            def l1wave(w):
                # strip-major issue: all 4 m-quadrant matmuls of strip 2w
                # run as soon as its gathers land, before strip 2w+1's
                ps1 = [
                    pspool.tile([128, 512], f32, tag="ps", name=f"ps1_{w}_{m}")
                    for m in range(4)
                ]
                for ih in range(2):
                    i = 2 * w + ih
                    for m in range(4):
                        nc.tensor.matmul(
                            out=ps1[m][:, 256 * ih : 256 * ih + 256],
                            lhsT=bd[32 * m : 32 * m + 32, 128 * i : 128 * i + 128],
                            rhs=stageX[32 * m : 32 * m + 32, B * i : B * i + B],
                            start=True,
                            stop=True,
                            tile_position=(32 * m, 0),
                        )
                for m in range(4):
                    # h1s quadrant t = 4m+i at cols 256t
                    evac(h1s[:, 1024 * m + 512 * w : 1024 * m + 512 * w + 512],
                         ps1[m][:, :], nk[0])
                    nk[0] += 1

            R = _CACHE.get("f32r", False)

            def cast(ap):
                return ap.bitcast(f32r) if R else ap

            def l23(wt, hin, hout, tp):
                ps2 = pspool.tile([128, 512], f32, tag="ps", name=f"ps_{nk[0]}")
                for ih in range(2):
                    t = 2 * tp + ih
                    nc.tensor.matmul(
                        out=ps2[:, 256 * ih : 256 * ih + 256],
                        lhsT=cast(wt[:, 128 * t : 128 * t + 128]),
                        rhs=cast(hin[:, B * t : B * t + B]),
                        start=True,
                        stop=True,
                        tile_position=(0, 0),
                    )
                evac(hout[:, 512 * tp : 512 * tp + 512], ps2[:, :], nk[0])
                nk[0] += 1

            def l4quad(i):
                # 4 concurrent col-group matmuls (one per m) for strip i
                for m in range(4):
                    t = 4 * m + i
                    nc.tensor.matmul(
                        out=psy[32 * m : 32 * m + 4, 256 * i : 256 * i + 256],
                        lhsT=w4t[:, 4 * t : 4 * t + 4],
                        rhs=h3s[:, B * t : B * t + B],
                        start=True,
                        stop=True,
                        tile_position=(0, 32 * m),
                    )

            A_PAIRS = [0, 2, 4, 6]  # quadrant pairs fed by L1 wave A (i=0,1)
            B_PAIRS = [1, 3, 5, 7]
            l1wave(0)
            for tp in A_PAIRS:
                l23(w2bd, h1s, h2s, tp)
            l1wave(1)
            for tp in B_PAIRS:
                l23(w2bd, h1s, h2s, tp)
            # L3: A pairs produce h3 quadrants with t%4 in {0,1} (strips 0,1)
            for tp in A_PAIRS:
                l23(w3bd, h2s, h3s, tp)
            l23(w3bd, h2s, h3s, 1)
            l4quad(0)
            l4quad(1)
            nc.scalar.activation(out=ys[:, 0:512], in_=psy[:, 0:512], func=SIGN)
            for tp in (3, 5, 7):
                l23(w3bd, h2s, h3s, tp)
            l4quad(2)
            l4quad(3)
            nc.scalar.activation(out=ys[:, 512:1024], in_=psy[:, 512:1024], func=SIGN)

            if DBG:
                nc.sync.dma_start(dbg_idx8[:], idx8[:])
                nc.sync.dma_start(dbg_offx[:], offx[:])
                nc.sync.dma_start(dbg_bd[:], bd[:])
                nc.sync.dma_start(dbg_sx[:], stageX[:])
                nc.sync.dma_start(dbg_h1s[:], h1s[:])

            # outS[16i+4m+j, b] = ys[32m+j, 256i+b]; one DMA per
            # (m, strip-half) so half A ships while half B computes
            for h in range(2):
                for m in range(4):
                    eng = nc.sync if m % 2 == 0 else nc.scalar
                    eng.dma_start(
                        out=outS[32 * h : 32 * h + 32, :].rearrange(
                            "(i mm j) b -> mm j i b", mm=4, j=4
                        )[m : m + 1],
                        in_=ys[32 * m : 32 * m + 4, 512 * h : 512 * h + 512].rearrange(
                            "j (i b) -> j i b", b=B
                        ),
                    )

    nc.compile()
    return nc


def _prep_core(c, inputs, mask, W1, W2, W3, W4, o_of_r):
    sl = slice(c * OSH, (c + 1) * OSH)
    mask_c = mask[sl]
    W1c, W2c, W3c, W4c = W1[sl], W2[sl], W3[sl], W4[sl]

    maskS = np.ascontiguousarray(mask_c[o_of_r])
    oconst = (o_of_r.astype(np.uint32) * np.uint32(IN))[:, None]

    # w1fp[o*IN + f, 32*(o%4) + h] = W1c[o, f, h]
    w1fp = np.zeros((OSH, IN, 4, HID), np.float32)
    o = np.arange(OSH)
    w1fp[o, :, o % 4, :] = W1c
    w1fp = w1fp.reshape(OSH * IN, 4 * HID)

    # block-diag L2/L3 weights: col block t=4m+i holds lhsT for quadrant t:
    #   w2bd[32j+h, 128t + 32j+k] = W2c[o(t,j), h, k]
    #   w3bd[32j+k, 128t + 32j+l] = W3c[o,k,l] * |W4c[o,l]|
    w4v = W4c[:, :, 0]  # [64, 32]
    w3p = W3c * np.abs(w4v)[:, None, :]
    w2bd = np.zeros((128, 2048), np.float32)
    w3bd = np.zeros((128, 2048), np.float32)
    sgn = np.sign(w4v).astype(np.float32)
    w4sgn = np.zeros((128, 64), np.float32)
    for o in range(OSH):
        i, m, j = o // 16, (o % 16) // 4, o % 4
        t = 4 * m + i
        w2bd[32 * j : 32 * j + 32, 128 * t + 32 * j : 128 * t + 32 * j + 32] = W2c[o]
        w3bd[32 * j : 32 * j + 32, 128 * t + 32 * j : 128 * t + 32 * j + 32] = w3p[o]
        w4sgn[32 * j : 32 * j + 32, 4 * t + j] = sgn[o]

    return {
        "maskS": maskS.astype(np.float32),
        "oconst": oconst,
        "w1fp": w1fp,
        "w2bd": w2bd,
        "w3bd": w3bd,
        "w4sgn": w4sgn,
    }


def kernel(inputs, mask, W1, W2, W3, W4, _run_kwargs=None):
    from concourse.bass_utils import run_bass_kernel_spmd

    inputs = np.asarray(inputs, np.float32)
    mask = np.asarray(mask, np.float32)
    W1 = np.asarray(W1, np.float32)
    W2 = np.asarray(W2, np.float32)
    W3 = np.asarray(W3, np.float32)
    W4 = np.asarray(W4, np.float32)

    if "nc" not in _CACHE:
        _CACHE["nc"] = _build_program()
    nc = _CACHE["nc"]

    o_of_r = _maps()
    xT = np.zeros((IN + 1, B), np.float32)
    xT[:IN] = inputs.T
    in_maps = []
    for c in range(NCORES):
        m = _prep_core(c, inputs, mask, W1, W2, W3, W4, o_of_r)
        m["xT"] = xT
        in_maps.append(m)

    kw = dict(_run_kwargs or {})
    res = run_bass_kernel_spmd(nc, in_maps, core_ids=list(range(NCORES)), **kw)
    out = np.concatenate([r["outS"].T for r in res.results], axis=1)
    if _run_kwargs is not None:
        _CACHE["last_result"] = res
    return out.astype(np.float32)


# revision 22
# speedup vs baseline: 1.3846x; 1.0067x over previous
"""Trainium2 Bass kernel for nn_BLayer_63780264346268 (topk_masking).

Math (per output unit o of 512):
  idx = top6(mask[o])                                  (6 of 1024 input features)
  h1 = relu(x[:, idx] @ W1[o, idx, :])                 (B,6)@(6,32)
  h2 = relu(h1 @ W2[o]); h3 = relu(h2 @ W3[o])         (B,32)@(32,32)
  y  = sigmoid(h3 @ W4[o]); q = (y>=.5)*2-1  == sign(h3 @ W4[o]) (as +/-1)

Distribution: 512 output units sharded across 8 cores (64 each). Top-k,
gathers and all math run on device; host does layout prep + final concat.

Per-core layout (64 units; o = 16i + 4m + j, i=strip, m=row-quadrant,
j=unit-in-quadrant; j1=j%2, j0=j//2):
  - top-8 values+indices per unit via DVE max/max_index (u32 indices);
    per-strip flatten DMAs build [128,4] u32 offset tiles (partition
    32m+8j+j6, col i).
  - ONE indirect DMA with multi-offsets builds the W1 block-diagonal lhsT
    (rows of a host-padded w1fp [65536, 128] where row o*1024+f holds
    W1[o,f,:] in column block 32*(o%4)); ONE builds stageX from xT rows
    (row 1024 = zeros kills the j6=6,7 padding slots).
  - L1: per (m,i) one (K=32, M=128, N=256) matmul, 4 m-quadrants issued
    back-to-back on distinct PE row-groups (tile_position=(32m,0)).
  - L2/L3: block-diagonal (K=128, M=128, N=256) per quadrant t=4m+i.
  - L4: (K=128, M=4, N=256) per t on PE col-group m; Sign activation
    (W3 pre-scaled by |W4|, W4 reduced to signs) = the binarize.
  - PE is pre-warmed with dummy matmuls during the topk/gather head so the
    HAM clock gate sits at 2.4 GHz when the real matmuls arrive.
"""

import numpy as np

OUT, IN, HID, B = 512, 1024, 32, 256
NCORES = 8
OSH = OUT // NCORES  # 64 output units per core
WARM_MMS = 30  # PE warm-up matmuls riding the topk/gather head
MULTI_OFF = False  # HW probe: multi-offset indirect only honors partition 0

_CACHE = {}


def _maps():
    # o = 16i + 4m + j ; mask/topk row r = 16i + 4m + j (identity)
    return np.arange(OSH)


def _build_program():
    import concourse.bacc as bacc
    import concourse.bass as bass
    import concourse.mybir as mybir
    import concourse.tile as tile

    f32 = mybir.dt.float32
    f32r = mybir.dt.float32r
    u32 = mybir.dt.uint32
    RELU = mybir.ActivationFunctionType.Relu
    SIGN = mybir.ActivationFunctionType.Sign

    nc = bacc.Bacc(None, target_bir_lowering=False, debug=False)

    xT = nc.dram_tensor("xT", [IN + 1, B], f32, kind="ExternalInput")
    maskS = nc.dram_tensor("maskS", [OSH, IN], f32, kind="ExternalInput")
    oconst = nc.dram_tensor("oconst", [OSH, 1], u32, kind="ExternalInput")
    w1fp = nc.dram_tensor("w1fp", [OSH * IN, 4 * HID], f32, kind="ExternalInput")
    w2bd_d = nc.dram_tensor("w2bd", [128, 2048], f32, kind="ExternalInput")
    w3bd_d = nc.dram_tensor("w3bd", [128, 2048], f32, kind="ExternalInput")
    w4sgn = nc.dram_tensor("w4sgn", [128, 64], f32, kind="ExternalInput")
    outS = nc.dram_tensor("outS", [OSH, B], f32, kind="ExternalOutput")
    DBG = _CACHE.get("debug", False)
    if DBG:
        dbg_idx8 = nc.dram_tensor("dbg_idx8", [OSH, 8], u32, kind="ExternalOutput")
        dbg_offx = nc.dram_tensor("dbg_offx", [128, 4], u32, kind="ExternalOutput")
        dbg_bd = nc.dram_tensor("dbg_bd", [128, 512], f32, kind="ExternalOutput")
        dbg_sx = nc.dram_tensor("dbg_sx", [128, 1024], f32, kind="ExternalOutput")
        dbg_h1s = nc.dram_tensor("dbg_h1s", [128, 4096], f32, kind="ExternalOutput")

    with tile.TileContext(nc) as tc:
        with (
            tc.tile_pool(name="const", bufs=1) as cpool,
            tc.tile_pool(name="psw", bufs=1, space="PSUM") as pswarm,
            tc.tile_pool(name="ps", bufs=5, space="PSUM") as pspool,
            tc.tile_pool(name="psy", bufs=1, space="PSUM") as psypool,
        ):
            # --- PE warm-up: dummy matmuls with no upstream deps ---
            warm = cpool.tile([128, 256], f32)
            nc.vector.memset(warm[:], 0.0)
            psw = pswarm.tile([128, 256], f32)
            for k in range(WARM_MMS):
                nc.tensor.matmul(
                    out=psw[:, 0:256],
                    lhsT=warm[:, 0:128],
                    rhs=warm[:, 0:256],
                    start=True,
                    stop=True,
                    tile_position=(0, 0),
                )

            # --- loads ---
            mask_t = cpool.tile([OSH, IN], f32)
            nc.sync.dma_start(mask_t[0:32, :], maskS[0:32, :])
            nc.scalar.dma_start(mask_t[32:64, :], maskS[32:64, :])
            oconst_t = cpool.tile([OSH, 1], u32)
            nc.sync.dma_start(oconst_t[:], oconst[:])
            w4t = cpool.tile([128, 64], f32)
            nc.scalar.dma_start(w4t[:], w4sgn[:])
            w2bd = cpool.tile([128, 2048], f32)
            nc.scalar.dma_start(w2bd[:], w2bd_d[:])
            w3bd = cpool.tile([128, 2048], f32)
            nc.scalar.dma_start(w3bd[:], w3bd_d[:])

            # --- top-8 values + indices per unit (6 real, 2 padding) ---
            mx8 = cpool.tile([OSH, 8], f32)
            idx8 = cpool.tile([OSH, 8], u32)
            nc.vector.max(out=mx8[:], in_=mask_t[:])
            nc.vector.max_index(out=idx8[:], in_max=mx8[:], in_values=mask_t[:])

            # gx/gw [64, 8] u32: per-unit-row x-row idx (pads -> zero row
            # IN) and w1fp row idx o*1024 + idx.
            gx = cpool.tile([OSH, 8], u32)
            gw = cpool.tile([OSH, 8], u32)
            nc.vector.tensor_copy(gx[:, 0:6], idx8[:, 0:6])
            nc.vector.memset(gx[:, 6:8], IN)
            nc.vector.tensor_tensor(
                out=gw[:],
                in0=idx8[:],
                in1=oconst_t[:].to_broadcast([OSH, 8]),
                op=mybir.AluOpType.add,
            )

            # offset tiles [128, 1] per strip: off_i[32m+8j+j6] <-
            # g_[16i+4m+j, j6]; separate tiles keep the indirect-DMA
            # dependencies per-strip.
            offx = [
                cpool.tile([128, 1], u32, name=f"offx_{i}", tag=f"offx_{i}")
                for i in range(4)
            ]
            offw = [
                cpool.tile([128, 1], u32, name=f"offw_{i}", tag=f"offw_{i}")
                for i in range(4)
            ]
            # each [16,8]->[128,1] flatten is 128 single-word descriptors;
            # split halves across both HWDGE queues to halve the SDMA drain
            for i in range(4):
                for h in range(2):
                    e1 = nc.sync if h == 0 else nc.scalar
                    e2 = nc.scalar if h == 0 else nc.sync
                    e1.dma_start(
                        out=offx[i][64 * h : 64 * h + 64, :],
                        in_=gx[16 * i + 8 * h : 16 * i + 8 * h + 8, 0:8],
                    )
                    e2.dma_start(
                        out=offw[i][64 * h : 64 * h + 64, :],
                        in_=gw[16 * i + 8 * h : 16 * i + 8 * h + 8, 0:8],
                    )

            # --- indirect row gathers: W -> bd block-diag lhsT; X -> stageX
            bd = cpool.tile([128, 512], f32)
            stageX = cpool.tile([128, 1024], f32)

            def xgather(i):
                nc.gpsimd.indirect_dma_start(
                    out=stageX[:, B * i : B * i + B],
                    out_offset=None,
                    in_=xT[:, :],
                    in_offset=bass.IndirectOffsetOnAxis(ap=offx[i][:], axis=0),
                )

            def wgather(i):
                nc.gpsimd.indirect_dma_start(
                    out=bd[:, 128 * i : 128 * i + 128],
                    out_offset=None,
                    in_=w1fp[:, :],
                    in_offset=bass.IndirectOffsetOnAxis(ap=offw[i][:], axis=0),
                )

            for i in range(4):
                xgather(i)
                wgather(i)

            def evac(dst, src, k):
                # alternate relu evacuation between Scalar and Vector
                if k % 2 == 0:
                    nc.scalar.activation(out=dst, in_=src, func=RELU)
                else:
                    nc.vector.tensor_scalar_max(dst, src, 0.0)

            # --- L1 (per (m,i): K=32, M=128, N=256 on PE row-quadrant m)
            # interleaved with L2/L3 block-diag quadrant matmuls so the PE
            # stream never stalls on the strip-2/3 gathers; L4 matmuls ride
            # two pairs behind L3. ---
            h1s = cpool.tile([128, 4096], f32)
            h2s = cpool.tile([128, 4096], f32)
            h3s = cpool.tile([128, 4096], f32)
            psy = psypool.tile([128, 1024], f32)
            nc.vector.memset(psy[:], 0.0)
            ys = cpool.tile([128, 1024], f32)
            nk = [0]

            def l1wave(w):
                # strip-major issue: all 4 m-quadrant matmuls of strip 2w
                # run as soon as its gathers land, before strip 2w+1's
                ps1 = [
                    pspool.tile([128, 512], f32, tag="ps", name=f"ps1_{w}_{m}")
                    for m in range(4)
                ]
                for ih in range(2):
                    i = 2 * w + ih
                    for m in range(4):
                        nc.tensor.matmul(
                            out=ps1[m][:, 256 * ih : 256 * ih + 256],
                            lhsT=bd[32 * m : 32 * m + 32, 128 * i : 128 * i + 128],
                            rhs=stageX[32 * m : 32 * m + 32, B * i : B * i + B],
                            start=True,
                            stop=True,
                            tile_position=(32 * m, 0),
                        )
                for m in range(4):
                    # h1s quadrant t = 4m+i at cols 256t
                    evac(h1s[:, 1024 * m + 512 * w : 1024 * m + 512 * w + 512],
                         ps1[m][:, :], nk[0])
                    nk[0] += 1

            R = _CACHE.get("f32r", False)

            def cast(ap):
                return ap.bitcast(f32r) if R else ap

            def l23(wt, hin, hout, tp):
                ps2 = pspool.tile([128, 512], f32, tag="ps", name=f"ps_{nk[0]}")
                for ih in range(2):
                    t = 2 * tp + ih
                    nc.tensor.matmul(
                        out=ps2[:, 256 * ih : 256 * ih + 256],
                        lhsT=cast(wt[:, 128 * t : 128 * t + 128]),
                        rhs=cast(hin[:, B * t : B * t + B]),
                        start=True,
                        stop=True,
                        tile_position=(0, 0),
                    )
                evac(hout[:, 512 * tp : 512 * tp + 512], ps2[:, :], nk[0])
                nk[0] += 1

            def l4quad(i):
                # 4 concurrent col-group matmuls (one per m) for strip i
                for m in range(4):
                    t = 4 * m + i
                    nc.tensor.matmul(
                        out=psy[32 * m : 32 * m + 4, 256 * i : 256 * i + 256],
                        lhsT=w4t[:, 4 * t : 4 * t + 4],
                        rhs=h3s[:, B * t : B * t + B],
                        start=True,
                        stop=True,
                        tile_position=(0, 32 * m),
                    )

            A_PAIRS = [0, 2, 4, 6]  # quadrant pairs fed by L1 wave A (i=0,1)
            B_PAIRS = [1, 3, 5, 7]
            l1wave(0)
            for tp in A_PAIRS:
                l23(w2bd, h1s, h2s, tp)
            l1wave(1)
            for tp in B_PAIRS:
                l23(w2bd, h1s, h2s, tp)
            # L3: A pairs produce h3 quadrants with t%4 in {0,1} (strips 0,1)
            for tp in A_PAIRS:
                l23(w3bd, h2s, h3s, tp)
            l23(w3bd, h2s, h3s, 1)
            l4quad(0)
            l4quad(1)
            nc.scalar.activation(out=ys[:, 0:512], in_=psy[:, 0:512], func=SIGN)
            for tp in (3, 5, 7):
                l23(w3bd, h2s, h3s, tp)
            l4quad(2)
            l4quad(3)
            nc.scalar.activation(out=ys[:, 512:1024], in_=psy[:, 512:1024], func=SIGN)

            if DBG:
                nc.sync.dma_start(dbg_idx8[:], idx8[:])
                nc.sync.dma_start(dbg_offx[:], offx[:])
                nc.sync.dma_start(dbg_bd[:], bd[:])
                nc.sync.dma_start(dbg_sx[:], stageX[:])
                nc.sync.dma_start(dbg_h1s[:], h1s[:])

            # outS[16i+4m+j, b] = ys[32m+j, 256i+b]; one DMA per
            # (m, strip-half) so half A ships while half B computes
            for h in range(2):
                for m in range(4):
                    eng = nc.sync if m % 2 == 0 else nc.scalar
                    eng.dma_start(
                        out=outS[32 * h : 32 * h + 32, :].rearrange(
                            "(i mm j) b -> mm j i b", mm=4, j=4
                        )[m : m + 1],
                        in_=ys[32 * m : 32 * m + 4, 512 * h : 512 * h + 512].rearrange(
                            "j (i b) -> j i b", b=B
                        ),
                    )

    nc.compile()
    return nc


def _prep_core(c, inputs, mask, W1, W2, W3, W4, o_of_r):
    sl = slice(c * OSH, (c + 1) * OSH)
    mask_c = mask[sl]
    W1c, W2c, W3c, W4c = W1[sl], W2[sl], W3[sl], W4[sl]

    maskS = np.ascontiguousarray(mask_c[o_of_r])
    oconst = (o_of_r.astype(np.uint32) * np.uint32(IN))[:, None]

    # w1fp[o*IN + f, 32*(o%4) + h] = W1c[o, f, h]
    w1fp = np.zeros((OSH, IN, 4, HID), np.float32)
    o = np.arange(OSH)
    w1fp[o, :, o % 4, :] = W1c
    w1fp = w1fp.reshape(OSH * IN, 4 * HID)

    # block-diag L2/L3 weights: col block t=4m+i holds lhsT for quadrant t:
    #   w2bd[32j+h, 128t + 32j+k] = W2c[o(t,j), h, k]
    #   w3bd[32j+k, 128t + 32j+l] = W3c[o,k,l] * |W4c[o,l]|
    w4v = W4c[:, :, 0]  # [64, 32]
    w3p = W3c * np.abs(w4v)[:, None, :]
    w2bd = np.zeros((128, 2048), np.float32)
    w3bd = np.zeros((128, 2048), np.float32)
    sgn = np.sign(w4v).astype(np.float32)
    w4sgn = np.zeros((128, 64), np.float32)
    for o in range(OSH):
        i, m, j = o // 16, (o % 16) // 4, o % 4
        t = 4 * m + i
        w2bd[32 * j : 32 * j + 32, 128 * t + 32 * j : 128 * t + 32 * j + 32] = W2c[o]
        w3bd[32 * j : 32 * j + 32, 128 * t + 32 * j : 128 * t + 32 * j + 32] = w3p[o]
        w4sgn[32 * j : 32 * j + 32, 4 * t + j] = sgn[o]

    return {
        "maskS": maskS.astype(np.float32),
        "oconst": oconst,
        "w1fp": w1fp,
        "w2bd": w2bd,
        "w3bd": w3bd,
        "w4sgn": w4sgn,
    }


def kernel(inputs, mask, W1, W2, W3, W4, _run_kwargs=None):
    from concourse.bass_utils import run_bass_kernel_spmd

    inputs = np.asarray(inputs, np.float32)
    mask = np.asarray(mask, np.float32)
    W1 = np.asarray(W1, np.float32)
    W2 = np.asarray(W2, np.float32)
    W3 = np.asarray(W3, np.float32)
    W4 = np.asarray(W4, np.float32)

    if "nc" not in _CACHE:
        _CACHE["nc"] = _build_program()
    nc = _CACHE["nc"]

    o_of_r = _maps()
    xT = np.zeros((IN + 1, B), np.float32)
    xT[:IN] = inputs.T
    in_maps = []
    for c in range(NCORES):
        m = _prep_core(c, inputs, mask, W1, W2, W3, W4, o_of_r)
        m["xT"] = xT
        in_maps.append(m)

    kw = dict(_run_kwargs or {})
    res = run_bass_kernel_spmd(nc, in_maps, core_ids=list(range(NCORES)), **kw)
    out = np.concatenate([r["outS"].T for r in res.results], axis=1)
    if _run_kwargs is not None:
        _CACHE["last_result"] = res
    return out.astype(np.float32)


# revision 23
# speedup vs baseline: 1.4108x; 1.0190x over previous
"""Trainium2 Bass kernel for nn_BLayer_63780264346268 (topk_masking).

Math (per output unit o of 512):
  idx = top6(mask[o])                                  (6 of 1024 input features)
  h1 = relu(x[:, idx] @ W1[o, idx, :])                 (B,6)@(6,32)
  h2 = relu(h1 @ W2[o]); h3 = relu(h2 @ W3[o])         (B,32)@(32,32)
  y  = sigmoid(h3 @ W4[o]); q = (y>=.5)*2-1  == sign(h3 @ W4[o]) (as +/-1)

Distribution: 512 output units sharded across 8 cores (64 each). Top-k,
gathers and all math run on device; host does layout prep + final concat.

Per-core layout (64 units; o = 16i + 4m + j, i=strip, m=row-quadrant,
j=unit-in-quadrant; j1=j%2, j0=j//2):
  - top-8 values+indices per unit via DVE max/max_index (u32 indices);
    per-strip flatten DMAs build [128,4] u32 offset tiles (partition
    32m+8j+j6, col i).
  - ONE indirect DMA with multi-offsets builds the W1 block-diagonal lhsT
    (rows of a host-padded w1fp [65536, 128] where row o*1024+f holds
    W1[o,f,:] in column block 32*(o%4)); ONE builds stageX from xT rows
    (row 1024 = zeros kills the j6=6,7 padding slots).
  - L1: per (m,i) one (K=32, M=128, N=256) matmul, 4 m-quadrants issued
    back-to-back on distinct PE row-groups (tile_position=(32m,0)).
  - L2/L3: block-diagonal (K=128, M=128, N=256) per quadrant t=4m+i.
  - L4: (K=128, M=4, N=256) per t on PE col-group m; Sign activation
    (W3 pre-scaled by |W4|, W4 reduced to signs) = the binarize.
  - PE is pre-warmed with dummy matmuls during the topk/gather head so the
    HAM clock gate sits at 2.4 GHz when the real matmuls arrive.
"""

import numpy as np

OUT, IN, HID, B = 512, 1024, 32, 256
NCORES = 8
OSH = OUT // NCORES  # 64 output units per core
WARM_MMS = 26  # PE warm-up matmuls riding the topk/gather head
MULTI_OFF = False  # HW probe: multi-offset indirect only honors partition 0

_CACHE = {}


def _maps():
    # o = 16i + 4m + j ; mask/topk row r = 16i + 4m + j (identity)
    return np.arange(OSH)


def _build_program():
    import concourse.bacc as bacc
    import concourse.bass as bass
    import concourse.mybir as mybir
    import concourse.tile as tile

    f32 = mybir.dt.float32
    f32r = mybir.dt.float32r
    u32 = mybir.dt.uint32
    RELU = mybir.ActivationFunctionType.Relu
    SIGN = mybir.ActivationFunctionType.Sign

    nc = bacc.Bacc(None, target_bir_lowering=False, debug=False)

    xT = nc.dram_tensor("xT", [IN + 1, B], f32, kind="ExternalInput")
    maskS = nc.dram_tensor("maskS", [OSH, IN], f32, kind="ExternalInput")
    oconst = nc.dram_tensor("oconst", [OSH, 1], u32, kind="ExternalInput")
    w1fp = nc.dram_tensor("w1fp", [OSH * IN, 4 * HID], f32, kind="ExternalInput")
    w2bd_d = nc.dram_tensor("w2bd", [128, 2048], f32, kind="ExternalInput")
    w3bd_d = nc.dram_tensor("w3bd", [128, 2048], f32, kind="ExternalInput")
    w4sgn = nc.dram_tensor("w4sgn", [128, 64], f32, kind="ExternalInput")
    outS = nc.dram_tensor("outS", [OSH, B], f32, kind="ExternalOutput")
    DBG = _CACHE.get("debug", False)
    if DBG:
        dbg_idx8 = nc.dram_tensor("dbg_idx8", [OSH, 8], u32, kind="ExternalOutput")
        dbg_offx = nc.dram_tensor("dbg_offx", [128, 4], u32, kind="ExternalOutput")
        dbg_bd = nc.dram_tensor("dbg_bd", [128, 512], f32, kind="ExternalOutput")
        dbg_sx = nc.dram_tensor("dbg_sx", [128, 1024], f32, kind="ExternalOutput")
        dbg_h1s = nc.dram_tensor("dbg_h1s", [128, 4096], f32, kind="ExternalOutput")

    with tile.TileContext(nc) as tc:
        with (
            tc.tile_pool(name="const", bufs=1) as cpool,
            tc.tile_pool(name="ps", bufs=6, space="PSUM") as pspool,
            tc.tile_pool(name="psy", bufs=1, space="PSUM") as psypool,
        ):
            # --- PE warm-up: dummy matmuls with no upstream deps ---
            warm = cpool.tile([128, 256], f32)
            nc.vector.memset(warm[:], 0.0)
            psy = psypool.tile([128, 1024], f32)
            for k in range(WARM_MMS):
                nc.tensor.matmul(
                    out=psy[:, 0:256],
                    lhsT=warm[:, 0:128],
                    rhs=warm[:, 0:256],
                    start=True,
                    stop=True,
                    tile_position=(0, 0),
                )

            # --- loads ---
            mask_t = cpool.tile([OSH, IN], f32)
            nc.sync.dma_start(mask_t[0:32, :], maskS[0:32, :])
            nc.scalar.dma_start(mask_t[32:64, :], maskS[32:64, :])
            oconst_t = cpool.tile([OSH, 1], u32)
            nc.sync.dma_start(oconst_t[:], oconst[:])
            w4t = cpool.tile([128, 64], f32)
            nc.scalar.dma_start(w4t[:], w4sgn[:])
            w2bd = cpool.tile([128, 2048], f32)
            nc.scalar.dma_start(w2bd[:], w2bd_d[:])
            w3bd = cpool.tile([128, 2048], f32)
            nc.scalar.dma_start(w3bd[:], w3bd_d[:])

            # --- top-8 values + indices per unit (6 real, 2 padding) ---
            mx8 = cpool.tile([OSH, 8], f32)
            idx8 = cpool.tile([OSH, 8], u32)
            nc.vector.max(out=mx8[:], in_=mask_t[:])
            nc.vector.max_index(out=idx8[:], in_max=mx8[:], in_values=mask_t[:])

            # gx/gw [64, 8] u32: per-unit-row x-row idx (pads -> zero row
            # IN) and w1fp row idx o*1024 + idx.
            gx = cpool.tile([OSH, 8], u32)
            gw = cpool.tile([OSH, 8], u32)
            nc.vector.tensor_copy(gx[:, 0:6], idx8[:, 0:6])
            nc.vector.memset(gx[:, 6:8], IN)
            nc.vector.tensor_tensor(
                out=gw[:],
                in0=idx8[:],
                in1=oconst_t[:].to_broadcast([OSH, 8]),
                op=mybir.AluOpType.add,
            )

            # offset tiles [128, 1] per strip: off_i[32m+8j+j6] <-
            # g_[16i+4m+j, j6]; separate tiles keep the indirect-DMA
            # dependencies per-strip.
            offx = [
                cpool.tile([128, 1], u32, name=f"offx_{i}", tag=f"offx_{i}")
                for i in range(4)
            ]
            offw = [
                cpool.tile([128, 1], u32, name=f"offw_{i}", tag=f"offw_{i}")
                for i in range(4)
            ]
            # each [16,8]->[128,1] flatten is 128 single-word descriptors;
            # split halves across both HWDGE queues to halve the SDMA drain
            for i in range(4):
                for h in range(2):
                    e1 = nc.sync if h == 0 else nc.scalar
                    e2 = nc.scalar if h == 0 else nc.sync
                    e1.dma_start(
                        out=offx[i][64 * h : 64 * h + 64, :],
                        in_=gx[16 * i + 8 * h : 16 * i + 8 * h + 8, 0:8],
                    )
                    e2.dma_start(
                        out=offw[i][64 * h : 64 * h + 64, :],
                        in_=gw[16 * i + 8 * h : 16 * i + 8 * h + 8, 0:8],
                    )

            # --- indirect row gathers: W -> bd block-diag lhsT; X -> stageX
            bd = cpool.tile([128, 512], f32)
            stageX = cpool.tile([128, 1024], f32)

            def xgather(i):
                nc.gpsimd.indirect_dma_start(
                    out=stageX[:, B * i : B * i + B],
                    out_offset=None,
                    in_=xT[:, :],
                    in_offset=bass.IndirectOffsetOnAxis(ap=offx[i][:], axis=0),
                )

            def wgather(i):
                nc.gpsimd.indirect_dma_start(
                    out=bd[:, 128 * i : 128 * i + 128],
                    out_offset=None,
                    in_=w1fp[:, :],
                    in_offset=bass.IndirectOffsetOnAxis(ap=offw[i][:], axis=0),
                )

            for i in range(4):
                xgather(i)
                wgather(i)

            def evac(dst, src, k):
                # alternate relu evacuation between Scalar and Vector
                if k % 2 == 0:
                    nc.scalar.activation(out=dst, in_=src, func=RELU)
                else:
                    nc.vector.tensor_scalar_max(dst, src, 0.0)

            # --- L1 (per (m,i): K=32, M=128, N=256 on PE row-quadrant m)
            # interleaved with L2/L3 block-diag quadrant matmuls so the PE
            # stream never stalls on the strip-2/3 gathers; L4 matmuls ride
            # two pairs behind L3. ---
            h1s = cpool.tile([128, 4096], f32)
            h2s = cpool.tile([128, 4096], f32)
            h3s = cpool.tile([128, 4096], f32)
            nc.vector.memset(psy[:], 0.0)
            ys = cpool.tile([128, 1024], f32)
            nk = [0]

            def l1wave(w):
                # strip-major issue: all 4 m-quadrant matmuls of strip 2w
                # run as soon as its gathers land, before strip 2w+1's
                ps1 = [
                    pspool.tile([128, 512], f32, tag="ps", name=f"ps1_{w}_{m}")
                    for m in range(4)
                ]
                for ih in range(2):
                    i = 2 * w + ih
                    for m in range(4):
                        nc.tensor.matmul(
                            out=ps1[m][:, 256 * ih : 256 * ih + 256],
                            lhsT=bd[32 * m : 32 * m + 32, 128 * i : 128 * i + 128],
                            rhs=stageX[32 * m : 32 * m + 32, B * i : B * i + B],
                            start=True,
                            stop=True,
                            tile_position=(32 * m, 0),
                        )
                for m in range(4):
                    # h1s quadrant t = 4m+i at cols 256t
                    evac(h1s[:, 1024 * m + 512 * w : 1024 * m + 512 * w + 512],
                         ps1[m][:, :], nk[0])
                    nk[0] += 1

            R = _CACHE.get("f32r", False)

            def cast(ap):
                return ap.bitcast(f32r) if R else ap

            def l23(wt, hin, hout, tp):
                ps2 = pspool.tile([128, 512], f32, tag="ps", name=f"ps_{nk[0]}")
                for ih in range(2):
                    t = 2 * tp + ih
                    nc.tensor.matmul(
                        out=ps2[:, 256 * ih : 256 * ih + 256],
                        lhsT=cast(wt[:, 128 * t : 128 * t + 128]),
                        rhs=cast(hin[:, B * t : B * t + B]),
                        start=True,
                        stop=True,
                        tile_position=(0, 0),
                    )
                evac(hout[:, 512 * tp : 512 * tp + 512], ps2[:, :], nk[0])
                nk[0] += 1

            def l4quad(i):
                # 4 concurrent col-group matmuls (one per m) for strip i
                for m in range(4):
                    t = 4 * m + i
                    nc.tensor.matmul(
                        out=psy[32 * m : 32 * m + 4, 256 * i : 256 * i + 256],
                        lhsT=w4t[:, 4 * t : 4 * t + 4],
                        rhs=h3s[:, B * t : B * t + B],
                        start=True,
                        stop=True,
                        tile_position=(0, 32 * m),
                    )

            A_PAIRS = [0, 2, 4, 6]  # quadrant pairs fed by L1 wave A (i=0,1)
            B_PAIRS = [1, 3, 5, 7]
            l1wave(0)
            for tp in A_PAIRS:
                l23(w2bd, h1s, h2s, tp)
            l1wave(1)
            for tp in B_PAIRS:
                l23(w2bd, h1s, h2s, tp)
            # L3: A pairs produce h3 quadrants with t%4 in {0,1} (strips 0,1)
            for tp in A_PAIRS:
                l23(w3bd, h2s, h3s, tp)
            l23(w3bd, h2s, h3s, 1)
            l4quad(0)
            l4quad(1)
            nc.scalar.activation(out=ys[:, 0:512], in_=psy[:, 0:512], func=SIGN)
            for tp in (3, 5, 7):
                l23(w3bd, h2s, h3s, tp)
            l4quad(2)
            l4quad(3)
            nc.scalar.activation(out=ys[:, 512:1024], in_=psy[:, 512:1024], func=SIGN)

            if DBG:
                nc.sync.dma_start(dbg_idx8[:], idx8[:])
                nc.sync.dma_start(dbg_offx[:], offx[:])
                nc.sync.dma_start(dbg_bd[:], bd[:])
                nc.sync.dma_start(dbg_sx[:], stageX[:])
                nc.sync.dma_start(dbg_h1s[:], h1s[:])

            # outS[16i+4m+j, b] = ys[32m+j, 256i+b]; one DMA per
            # (m, strip-half) so half A ships while half B computes
            for h in range(2):
                for m in range(4):
                    eng = nc.sync if m % 2 == 0 else nc.scalar
                    eng.dma_start(
                        out=outS[32 * h : 32 * h + 32, :].rearrange(
                            "(i mm j) b -> mm j i b", mm=4, j=4
                        )[m : m + 1],
                        in_=ys[32 * m : 32 * m + 4, 512 * h : 512 * h + 512].rearrange(
                            "j (i b) -> j i b", b=B
                        ),
                    )

    nc.compile()
    return nc


def _prep_core(c, inputs, mask, W1, W2, W3, W4, o_of_r):
    sl = slice(c * OSH, (c + 1) * OSH)
    mask_c = mask[sl]
    W1c, W2c, W3c, W4c = W1[sl], W2[sl], W3[sl], W4[sl]

    maskS = np.ascontiguousarray(mask_c[o_of_r])
    oconst = (o_of_r.astype(np.uint32) * np.uint32(IN))[:, None]

    # w1fp[o*IN + f, 32*(o%4) + h] = W1c[o, f, h]
    w1fp = np.zeros((OSH, IN, 4, HID), np.float32)
    o = np.arange(OSH)
    w1fp[o, :, o % 4, :] = W1c
    w1fp = w1fp.reshape(OSH * IN, 4 * HID)

    # block-diag L2/L3 weights: col block t=4m+i holds lhsT for quadrant t:
    #   w2bd[32j+h, 128t + 32j+k] = W2c[o(t,j), h, k]
    #   w3bd[32j+k, 128t + 32j+l] = W3c[o,k,l] * |W4c[o,l]|
    w4v = W4c[:, :, 0]  # [64, 32]
    w3p = W3c * np.abs(w4v)[:, None, :]
    w2bd = np.zeros((128, 2048), np.float32)
    w3bd = np.zeros((128, 2048), np.float32)
    sgn = np.sign(w4v).astype(np.float32)
    w4sgn = np.zeros((128, 64), np.float32)
    for o in range(OSH):
        i, m, j = o // 16, (o % 16) // 4, o % 4
        t = 4 * m + i
        w2bd[32 * j : 32 * j + 32, 128 * t + 32 * j : 128 * t + 32 * j + 32] = W2c[o]
        w3bd[32 * j : 32 * j + 32, 128 * t + 32 * j : 128 * t + 32 * j + 32] = w3p[o]
        w4sgn[32 * j : 32 * j + 32, 4 * t + j] = sgn[o]

    return {
        "maskS": maskS.astype(np.float32),
        "oconst": oconst,
        "w1fp": w1fp,
        "w2bd": w2bd,
        "w3bd": w3bd,
        "w4sgn": w4sgn,
    }


def kernel(inputs, mask, W1, W2, W3, W4, _run_kwargs=None):
    from concourse.bass_utils import run_bass_kernel_spmd

    inputs = np.asarray(inputs, np.float32)
    mask = np.asarray(mask, np.float32)
    W1 = np.asarray(W1, np.float32)
    W2 = np.asarray(W2, np.float32)
    W3 = np.asarray(W3, np.float32)
    W4 = np.asarray(W4, np.float32)

    if "nc" not in _CACHE:
        _CACHE["nc"] = _build_program()
    nc = _CACHE["nc"]

    o_of_r = _maps()
    xT = np.zeros((IN + 1, B), np.float32)
    xT[:IN] = inputs.T
    in_maps = []
    for c in range(NCORES):
        m = _prep_core(c, inputs, mask, W1, W2, W3, W4, o_of_r)
        m["xT"] = xT
        in_maps.append(m)

    kw = dict(_run_kwargs or {})
    res = run_bass_kernel_spmd(nc, in_maps, core_ids=list(range(NCORES)), **kw)
    out = np.concatenate([r["outS"].T for r in res.results], axis=1)
    if _run_kwargs is not None:
        _CACHE["last_result"] = res
    return out.astype(np.float32)
